# revision 33
# baseline (speedup 1.0000x reference)
"""CrossAttnBlock on 8 trn2 NeuronCores.

Sharding: core c -> batch b=c//4, rank r=c%4 within the batch group.
Attention is Megatron-sliced over heads (4 of 16 per core); the
out-projection partial sums are combined with TWO chunked bf16
ReduceScatters (one per K-half), each launched as soon as its half of
the attention context is ready.  After the RS, core r owns k-rows
{r*128..+128} of each half; the host maps them back.

Key optimizations vs the 636us baseline:
- ALL transposes (q, kv, v-ctx, FFN x / h1) moved from DMA-transpose
  (5.7us each, serialized on one ring) to PE-array transposes (53ns per
  128x128 chunk) + one PSUM->SBUF copy per tile.  The PE p-state ramp
  (0.65/1.2/2.4 GHz) rewards a continuously-busy PE; the old prologue
  idled the PE for 300us waiting on the sync DMA ring.
- q/kv stream in as bf16 (halves input DMA; LN stats lose nothing vs
  the 2e-2 tolerance) and the LN normalize hits the DVE 4x mode.
- LN inv-std batched over tile PAIRS (the [128,1] ACT ops have a
  ~350-cycle fixed overhead) using Sqrt+DVE-reciprocal: Sqrt shares an
  ACT table set with the Identity/Copy drains, so the whole prologue
  takes ONE table load (Ln/Exp alternation was 40 loads = 51us).  The
  FFN LNs use a 3-step Newton rsqrt on the DVE (no ACT tables at all).
- The previous s-group's projection chains are emitted BETWEEN the next
  group's LN/transpose pairs so the PE never starves in the prologue.
- ReduceScatter payload in fp8e4 (partial sums ~N(0,0.5); the wire
  phase drops ~2us and staging DMA halves).
- exp(scores) stays on the scalar/ACT engine (the only table engine):
  128 x [128,1024] = hard ~141us floor; everything else in the
  attention phase is kept OFF the scalar engine so exp back-to-backs.
- Attention epilogue (denominators + out-proj) borrows the freed pv
  PSUM banks (per-tag rings) instead of the score ring, so the next
  K-half's QK->exp pipeline restarts immediately.
- DMA rings: Pool ring = bulk loads + rs_in staging + collectives (the
  rs_in stores precede their collective in ring order), SP ring = et
  stream + rs_out loads + output stores.  The ACT ring stays empty so
  exp dispatch is never stalled; q/kv loads stay off the SP ring, which
  doubles as the semaphore hub.
"""
import sys
import numpy as np

sys.path.insert(0, "/opt/trn_rl_repo")

import ml_dtypes  # noqa: E402
import concourse.bass as bass  # noqa: E402
import concourse.mybir as mybir  # noqa: E402
import concourse.tile as tile  # noqa: E402
from concourse import bacc  # noqa: E402
from concourse import bass_utils  # noqa: E402
from concourse.masks import make_identity  # noqa: E402

F32 = mybir.dt.float32
BF16 = mybir.dt.bfloat16
FP8 = mybir.dt.float8e4
AF = mybir.ActivationFunctionType
OP = mybir.AluOpType

D = 1024
H = 16
HD = 64
B = 2
K = 1024
S = 4096
EPS = 1e-5
N_CORES = 8
KQ = K // 4          # rows per core after the two ReduceScatters
HC = 4               # heads per core
DH = HC * HD         # ctx dims per core (256)
P = 128
DC = D // P          # 8 D-chunks
D2 = 2 * D

_CACHE = {}


def _ln_stats(nc, pool, xt, mv, eng=None):
    """bn stats of xt [128, 1024] into mv [128, 2] (mean, var)."""
    if eng is None:
        eng = nc.vector
    st = pool.tile([P, 2, 6], F32, tag="ln_st")
    xs = xt.rearrange("p (s f) -> p s f", s=2)
    for i in range(2):
        eng.bn_stats(out=st[:, i, :], in_=xs[:, i, :])
    eng.bn_aggr(out=mv, in_=st[:, :, :])


def _ln_norm(nc, pool, xt, mean_col, rs_col):
    xn = pool.tile([P, D], BF16, tag="ln_out")
    nc.vector.tensor_scalar(out=xn, in0=xt, scalar1=mean_col, scalar2=rs_col,
                            op0=OP.subtract, op1=OP.mult)
    return xn


def _inv_std(nc, pool, mv2, n, gate_cols=None):
    """inv-std for n tiles batched: mv2 [128, n, 2] -> rs [128, n].
    Sqrt on ACT (shares its table set with the Identity/Copy drains, so
    the whole prologue needs ONE table load) + reciprocal on DVE."""
    sq = pool.tile([P, n], F32, tag="ln_sq")
    nc.scalar.activation(sq, mv2[:, :, 1], AF.Sqrt, bias=nc._eps_t[:, :],
                         scale=1.0)
    rs = pool.tile([P, n], F32, tag="ln_rs")
    nc.vector.reciprocal(rs, sq)
    if gate_cols is not None:
        nc.vector.tensor_tensor(out=rs, in0=rs, in1=gate_cols, op=OP.mult)
    return rs


def _rsqrt_newton(nc, pool, v_col, n=1):
    """1/sqrt(v+eps) on DVE only (no ACT tables): seed y0=1/v, then 3
    Newton steps y <- y*(1.5 - 0.5*v*y^2).  Post-residual LN variance
    sits in ~[0.8, 1.6], where this converges to ~1e-4."""
    v = pool.tile([P, n], F32, tag="nw_v")
    nc.vector.tensor_scalar_add(v, v_col, nc._eps_t[:, 0:1])
    y = pool.tile([P, n], F32, tag="nw_y")
    nc.vector.reciprocal(y, v)
    hv = pool.tile([P, n], F32, tag="nw_hv")
    nc.vector.tensor_scalar_mul(hv, v, -0.5)
    for _ in range(3):
        y2 = pool.tile([P, n], F32, tag="nw_y2", bufs=2)
        nc.vector.tensor_tensor(out=y2, in0=y, in1=y, op=OP.mult)
        t = pool.tile([P, n], F32, tag="nw_t", bufs=2)
        nc.vector.tensor_scalar(out=t, in0=y2, scalar1=hv[:, 0:1],
                                scalar2=1.5, op0=OP.mult, op1=OP.add)
        yn = pool.tile([P, n], F32, tag="nw_yn", bufs=2)
        nc.vector.tensor_tensor(out=yn, in0=y, in1=t, op=OP.mult)
        y = yn
    return y


def _build_nc():
    nc = bacc.Bacc("TRN2", target_bir_lowering=False, debug=False,
                   num_devices=N_CORES)

    def din(name, shape, dt=F32):
        return nc.dram_tensor(name, shape, dt, kind="ExternalInput")

    q_d = din("q", [K, D], BF16)
    kv_d = din("kv", [S, D], BF16)
    et_d = din("et", [S, K], BF16)
    gate_d = din("gate", [K, 1])
    qres_d = din("q_res", [KQ, D])
    wq_d = din("wq", [D, DH], BF16)
    wk_d = din("wk", [D, DH], BF16)
    wv_d = din("wv", [D, DH], BF16)
    bq_d = din("bq", [1, DH], BF16)
    bk_d = din("bk", [DH, 1])
    bv_d = din("bv", [DH, 1])
    grow_d = din("growb", [1, K], BF16)
    wo_d = din("wo", [DH, D], BF16)
    w1_d = din("w1", [D, D2], BF16)
    b1_d = din("b1", [1, D2], BF16)
    w2_d = din("w2", [D2, D], BF16)
    b2_d = din("b2", [1, D], BF16)
    out_d = nc.dram_tensor("xq", [KQ, D], F32, kind="ExternalOutput")

    rs_out = [nc.dram_tensor(f"rs_out{i}", [P, D], FP8) for i in range(2)]
    groups = [[0, 1, 2, 3], [4, 5, 6, 7]]

    with tile.TileContext(nc) as tc:
        with (
            tc.tile_pool(name="const", bufs=1) as cpool,
            tc.tile_pool(name="persist", bufs=1) as pp,
            tc.tile_pool(name="att", bufs=6) as apool,
            tc.tile_pool(name="ets", bufs=6) as espool,
            tc.tile_pool(name="dram", bufs=1, space="DRAM") as dpool,
        ):
            # ---- constants ----
            eps_t = cpool.tile([P, 1], F32)
            nc.vector.memset(eps_t, EPS)
            nc._eps_t = eps_t
            ident = cpool.tile([P, P], BF16)
            make_identity(nc, ident)
            ones_row = cpool.tile([1, 512], BF16)
            nc.vector.memset(ones_row, 1.0)
            ones64 = cpool.tile([P, 64], BF16)
            nc.vector.memset(ones64, 1.0)
            gsb = cpool.tile([P, DC], F32)
            nc.sync.dma_start(out=gsb, in_=gate_d.ap().rearrange(
                "(t p) o -> p (t o)", p=P))
            grow_bf = cpool.tile([1, K], BF16)
            nc.sync.dma_start(out=grow_bf, in_=grow_d[:, :])
            bq_bf = cpool.tile([1, DH], BF16)
            nc.sync.dma_start(out=bq_bf, in_=bq_d[:, :])
            bk_col = cpool.tile([P, 2], F32)
            nc.sync.dma_start(out=bk_col, in_=bk_d.ap().rearrange(
                "(c p) o -> p (c o)", p=P))
            bv_col = cpool.tile([P, 2], F32)
            nc.sync.dma_start(out=bv_col, in_=bv_d.ap().rearrange(
                "(c p) o -> p (c o)", p=P))
            b1_bf = cpool.tile([1, D2], BF16)
            nc.sync.dma_start(out=b1_bf, in_=b1_d[:, :])
            b2_bf = cpool.tile([1, D], BF16)
            nc.sync.dma_start(out=b2_bf, in_=b2_d[:, :])

            # ---- persistent activation tensors ----
            qpT = pp.tile([P, 2, K], BF16)       # [2 heads x 64, hp, k]
            kpT = pp.tile([P, 2, S], BF16)
            vp = pp.tile([P, 32, HC * 65], BF16)  # [s%128, s//128, h*65+(hd|one)]
            ctxT = pp.tile([P, 2, K], BF16)

            # ============ q: LN+gate -> PE transpose -> projection ========
            with (
                tc.tile_pool(name="projw", bufs=1) as wpool,
                tc.tile_pool(name="psA", bufs=3, space="PSUM") as psA,
                tc.tile_pool(name="psT", bufs=3, space="PSUM") as psT,
                tc.tile_pool(name="psS0", bufs=1, space="PSUM") as psS0,
            ):
                wq_bf = wpool.tile([P, DC, DH], BF16)
                nc.gpsimd.dma_start(out=wq_bf, in_=wq_d.ap().rearrange(
                    "(c p) n -> p c n", p=P))
                wk_bf = wpool.tile([P, DC, DH], BF16)
                nc.gpsimd.dma_start(out=wk_bf, in_=wk_d.ap().rearrange(
                    "(c p) n -> p c n", p=P))
                wv_bf = wpool.tile([P, DC, DH], BF16)
                nc.gpsimd.dma_start(out=wv_bf, in_=wv_d.ap().rearrange(
                    "(c p) n -> p c n", p=P))

                def qk_exp_mult(kb, sc, spool):
                    # scores -> exp -> bias-multiply for one s-tile; the
                    # returned `at` tiles live in the ring until their PV
                    # matmuls consume them.
                    ksl = slice(kb * 512, (kb + 1) * 512)
                    et_blk = espool.tile([P, 512], BF16, tag="et")
                    nc.sync.dma_start(
                        out=et_blk,
                        in_=et_d.ap()[sc * P:(sc + 1) * P, ksl])
                    ats = []
                    for hp in range(2):
                        sps = spool.tile([P, 1024], F32, tag="sps")
                        nc.tensor.matmul(
                            sps[:, 0:512],
                            kpT[0:64, hp, sc * P:(sc + 1) * P],
                            qpT[0:64, hp, ksl],
                            start=True, stop=True, tile_position=(0, 0))
                        nc.tensor.matmul(
                            sps[:, 512:1024],
                            kpT[64:128, hp, sc * P:(sc + 1) * P],
                            qpT[64:128, hp, ksl],
                            start=True, stop=True, tile_position=(64, 0))
                        eq = apool.tile([P, 1024], BF16, tag="eq")
                        nc.scalar.activation(eq, sps[:, :], AF.Exp)
                        at = apool.tile([P, 1024], BF16, tag="at")
                        et_v = et_blk[:, :].rearrange(
                            "p (o f) -> p o f", o=1).broadcast_to(
                            [P, 2, 512])
                        nc.vector.tensor_tensor(
                            out=at[:, :].rearrange("p (o f) -> p o f", o=2),
                            in0=eq[:, :].rearrange("p (o f) -> p o f", o=2),
                            in1=et_v, op=OP.mult)
                        ats.append(at)
                    return ats

                def ln_pair(lpool, xts, gate_cols=None):
                    """LN a pair of loaded [128, D] tiles -> two bf16 tiles."""
                    mv2 = lpool.tile([P, 2, 2], F32, tag="ln_mv")
                    for i, xt in enumerate(xts):
                        _ln_stats(nc, lpool, xt, mv2[:, i, :])
                    rs2 = _inv_std(nc, lpool, mv2, 2, gate_cols)
                    return [
                        _ln_norm(nc, lpool, xt, mv2[:, i, 0:1], rs2[:, i:i + 1])
                        for i, xt in enumerate(xts)]

                def transp_tile(xn, dst, copy_eng):
                    tp = psT.tile([P, DC, P], BF16, tag="tp")
                    for dc in range(DC):
                        nc.tensor.transpose(
                            tp[:, dc, :], xn[:, dc * P:(dc + 1) * P], ident)
                    if copy_eng == "s":
                        nc.scalar.copy(dst, tp)
                    else:
                        nc.vector.tensor_copy(dst, tp)

                def q_proj_unit(hp, tb, qT):
                    # q projection chain: psum[2hd, 512 tok] (+rank-1 gate
                    # bias term)
                    ps = psA.tile([P, 512], F32, tag="mm")
                    for dc in range(DC):
                        nc.tensor.matmul(
                            ps[:, :],
                            wq_bf[:, dc, hp * P:(hp + 1) * P],
                            qT[:, dc, tb * 512:(tb + 1) * 512],
                            start=(dc == 0), stop=False)
                    nc.tensor.matmul(
                        ps[:, :], bq_bf[0:1, hp * P:(hp + 1) * P],
                        grow_bf[0:1, tb * 512:(tb + 1) * 512],
                        start=False, stop=True)
                    nc.scalar.activation(
                        qpT[:, hp, tb * 512:(tb + 1) * 512], ps[:, :],
                        AF.Identity)

                def kv_proj_unit(sg, hp, sb_, which, kvT, vpT):
                    # one k- or v-projection chain (bias folded into drain)
                    ssl = slice(sb_ * 512, (sb_ + 1) * 512)
                    w_bf = wk_bf if which == "k" else wv_bf
                    ps = psA.tile([P, 512], F32, tag="mm")
                    for dc in range(DC):
                        nc.tensor.matmul(
                            ps[:, :],
                            w_bf[:, dc, hp * P:(hp + 1) * P],
                            kvT[:, dc, ssl],
                            start=(dc == 0), stop=(dc == DC - 1))
                    if which == "k":
                        osl = slice(sg * 1024 + sb_ * 512,
                                    sg * 1024 + (sb_ + 1) * 512)
                        nc.scalar.activation(
                            kpT[:, hp, osl], ps[:, :], AF.Identity,
                            bias=bk_col[:, hp:hp + 1], scale=1.0)
                    else:
                        nc.scalar.activation(
                            vpT[:, hp, ssl], ps[:, :], AF.Identity,
                            bias=bv_col[:, hp:hp + 1], scale=1.0)

                def vp_unit(sg, hp, vpT):
                    # vp natural layout [s, hd] per head (+ ones column):
                    # PE-transpose vpT chunks, then strided inserts.
                    tp = psT.tile([P, DC, P], BF16, tag="tp")
                    for dc in range(DC):
                        nc.tensor.transpose(
                            tp[:, dc, :],
                            vpT[:, hp, dc * P:(dc + 1) * P], ident)
                    for half in range(2):
                        h = hp * 2 + half
                        if half == 0:
                            nc.vector.tensor_copy(
                                vp[:, sg * 8:(sg + 1) * 8,
                                   h * 65:h * 65 + 64],
                                tp[:, :, half * 64:half * 64 + 64])
                        else:
                            nc.scalar.copy(
                                vp[:, sg * 8:(sg + 1) * 8,
                                   h * 65:h * 65 + 64],
                                tp[:, :, half * 64:half * 64 + 64])

                # q LN/transposes, then kv by s-group.  The previous
                # group's projection chains are emitted BETWEEN the next
                # group's LN/transpose pairs so the PE never starves (and
                # stays at full p-state) while the DVE runs LN stats.
                qT = wpool.tile([P, DC, K], BF16, tag="qT")
                with (
                    tc.tile_pool(name="lnq", bufs=4) as lpool,
                    tc.tile_pool(name="lnkv_big", bufs=2) as kbig,
                ):
                    for tp_ in range(4):
                        xts = []
                        for i in range(2):
                            t = tp_ * 2 + i
                            qt = lpool.tile([P, D], BF16, tag=f"ln_in{i}")
                            nc.gpsimd.dma_start(
                                out=qt, in_=q_d[t * P:(t + 1) * P, :])
                            xts.append(qt)
                        qns = ln_pair(lpool, xts,
                                      gate_cols=gsb[:, tp_ * 2:tp_ * 2 + 2])
                        for i in range(2):
                            t = tp_ * 2 + i
                            transp_tile(qns[i], qT[:, :, t * P:(t + 1) * P],
                                        "s")
                    for h in range(HC):
                        nc.vector.memset(vp[:, :, h * 65 + 64:h * 65 + 65], 1.0)
                    pending = [lambda hp=hp, tb=tb: q_proj_unit(hp, tb, qT)
                               for hp in range(2) for tb in range(2)]
                    for sg in range(4):
                        kvT = kbig.tile([P, DC, 1024], BF16, tag="kvT")
                        vpT = kbig.tile([P, 2, 1024], BF16, tag="vpT")
                        for tp_ in range(4):
                            xts = []
                            for i in range(2):
                                t = tp_ * 2 + i
                                st_ = sg * 1024 + t * P
                                xt = lpool.tile([P, D], BF16, tag=f"ln_in{i}")
                                nc.gpsimd.dma_start(
                                    out=xt, in_=kv_d[st_:st_ + P, :])
                                xts.append(xt)
                            xns = ln_pair(lpool, xts)
                            for i in range(2):
                                t = tp_ * 2 + i
                                transp_tile(
                                    xns[i], kvT[:, :, t * P:(t + 1) * P],
                                    "s")
                            n_emit = (len(pending) + 3 - tp_) // (4 - tp_)
                            for u in pending[:n_emit]:
                                u()
                            pending = pending[n_emit:]
                        assert not pending
                        pending = []
                        for hp in range(2):
                            for sb_ in range(2):
                                for which in ("k", "v"):
                                    pending.append(
                                        lambda sg=sg, hp=hp, sb_=sb_,
                                        which=which, kvT=kvT, vpT=vpT:
                                        kv_proj_unit(sg, hp, sb_, which,
                                                     kvT, vpT))
                            pending.append(
                                lambda sg=sg, hp=hp, vpT=vpT:
                                vp_unit(sg, hp, vpT))
                        # order: k/v chains for hp then its vp transpose
                    # Hoist kb0's first three score/exp/multiply tiles so
                    # the scalar engine starts the exp stream WHILE the PE
                    # drains sg3's projection chains (the hoisted QKs use a
                    # dedicated one-slot psum pool and are interleaved
                    # between the drain units so the in-order PE never
                    # blocks on the exp that frees the slot).
                    pre_ats0 = []
                    for i, u in enumerate(pending):
                        if i % 3 == 0 and len(pre_ats0) < 3:
                            pre_ats0.append(
                                qk_exp_mult(0, len(pre_ats0), psS0))
                        u()

            # tail weights: pool ring is idle once q/kv loads are done;
            # these loads overlap the start of attention.  The wt pool is
            # entered only now so its 76KB/partition comes from the freed
            # prologue zones instead of shrinking them.
            wt_cm = tc.tile_pool(name="wt", bufs=1)
            wt = wt_cm.__enter__()
            wo_bf = wt.tile([P, 2, D], BF16)
            nc.gpsimd.dma_start(out=wo_bf, in_=wo_d.ap().rearrange(
                "(c p) n -> p c n", p=P))
            w1_bf = wt.tile([P, DC, D2], BF16)
            nc.gpsimd.dma_start(out=w1_bf, in_=w1_d.ap().rearrange(
                "(c p) n -> p c n", p=P))
            w2_bf = wt.tile([P, D2 // P, D], BF16)
            nc.gpsimd.dma_start(out=w2_bf, in_=w2_d.ap().rearrange(
                "(c p) n -> p c n", p=P))
            qres_sb = wt.tile([P, 2, D], F32)
            nc.gpsimd.dma_start(out=qres_sb, in_=qres_d.ap().rearrange(
                "(t p) d -> p t d", p=P))
            x_sb = wt.tile([P, 2, D], F32)
            xfT = wt.tile([P, DC, KQ], BF16)

            # ======================= attention ==========================
            # loop kb (k halves) -> sc (s tiles) -> hp (head pairs);
            # the bias-exp block streams from HBM per (kb, sc).  After each
            # kb, the out-proj partial for that k-half is computed and its
            # ReduceScatter launched (overlapping the next kb / the FFN).
            with (
                tc.tile_pool(name="attr", bufs=2) as rpool,
                tc.tile_pool(name="ysb", bufs=1) as ypool,
                tc.tile_pool(name="psS", bufs=2, space="PSUM") as psS,
                tc.tile_pool(name="psPV", bufs=1, space="PSUM") as psPV,
            ):
                def _unused(kb, sc):
                    ksl = slice(kb * 512, (kb + 1) * 512)
                    et_blk = espool.tile([P, 512], BF16, tag="et")
                    nc.sync.dma_start(
                        out=et_blk,
                        in_=et_d.ap()[sc * P:(sc + 1) * P, ksl])
                    ats = []
                    for hp in range(2):
                        sps = psS.tile([P, 1024], F32, tag="sps")
                        nc.tensor.matmul(
                            sps[:, 0:512],
                            kpT[0:64, hp, sc * P:(sc + 1) * P],
                            qpT[0:64, hp, ksl],
                            start=True, stop=True, tile_position=(0, 0))
                        nc.tensor.matmul(
                            sps[:, 512:1024],
                            kpT[64:128, hp, sc * P:(sc + 1) * P],
                            qpT[64:128, hp, ksl],
                            start=True, stop=True, tile_position=(64, 0))
                        eq = apool.tile([P, 1024], BF16, tag="eq")
                        nc.scalar.activation(eq, sps[:, :], AF.Exp)
                        at = apool.tile([P, 1024], BF16, tag="at")
                        et_v = et_blk[:, :].rearrange(
                            "p (o f) -> p o f", o=1).broadcast_to(
                            [P, 2, 512])
                        nc.vector.tensor_tensor(
                            out=at[:, :].rearrange("p (o f) -> p o f", o=2),
                            in0=eq[:, :].rearrange("p (o f) -> p o f", o=2),
                            in1=et_v, op=OP.mult)
                        ats.append(at)
                    return ats

                pre_ats = pre_ats0
                for kb in range(K // 512):
                    ksl = slice(kb * 512, (kb + 1) * 512)
                    pvs = [psPV.tile([65, 512], F32, tag=f"pv{h}",
                                     name=f"pv_{kb}_{h}")
                           for h in range(HC)]
                    for sc in range(S // P):
                        if pre_ats is not None and sc < len(pre_ats):
                            ats = pre_ats[sc]
                        else:
                            ats = qk_exp_mult(kb, sc, psS)
                        for hp in range(2):
                            at = ats[hp]
                            he = hp * 2
                            ho = hp * 2 + 1
                            nc.tensor.matmul(
                                pvs[he][:, :],
                                vp[:, sc, he * 65:(he + 1) * 65],
                                at[:, 0:512],
                                start=(sc == 0), stop=(sc == S // P - 1))
                            nc.tensor.matmul(
                                pvs[ho][:, :],
                                vp[:, sc, ho * 65:(ho + 1) * 65],
                                at[:, 512:1024],
                                start=(sc == 0), stop=(sc == S // P - 1))
                    # prefetch the NEXT K-half's first scores/exp/multiply
                    # so the scalar engine keeps running through the
                    # epilogue below (its PE work sits ahead of the
                    # epilogue matmuls in the queue).
                    if kb == 0:
                        pre_ats = [qk_exp_mult(1, s, psS) for s in range(2)]
                    # denominators: spread the 4 heads onto partitions
                    # {0,32,64,96} so one reciprocal covers all of them;
                    # broadcast 1/den across 64 partitions with a 1-row PE
                    # matmul whose psum borrows a freed score-ring slot.
                    last = kb == K // 512 - 1
                    dall = rpool.tile([97, 512], F32, tag="dall")
                    for h in range(HC):
                        if h % 2 == 1:
                            nc.scalar.copy(dall[32 * h:32 * h + 1, :],
                                           pvs[h][64:65, :])
                        else:
                            nc.vector.tensor_copy(dall[32 * h:32 * h + 1, :],
                                                  pvs[h][64:65, :])
                    rden = rpool.tile([97, 512], BF16, tag="rden")
                    with nc.allow_low_precision(
                            reason="softmax denom reciprocal in bf16; "
                                   "ctx is bf16 anyway"):
                        nc.vector.reciprocal(rden, dall[:, :])
                    for h in range(HC):
                        pv = pvs[h]
                        pb = (h % 2) * 64
                        hp = h // 2
                        rps = psS.tile([64, 512], F32, tag="sps",
                                       name=f"rps_{kb}_{h}")
                        nc.tensor.matmul(rps[:, :],
                                         ones64[32 * h:32 * h + 1, :],
                                         rden[32 * h:32 * h + 1, :],
                                         start=True, stop=True,
                                         tile_position=(32 * h, 0))
                        rrs = rpool.tile([64, 512], BF16, tag="rrs")
                        if last and h % 2 == 1:
                            nc.scalar.copy(rrs, rps[:, :])
                        else:
                            nc.vector.tensor_copy(rrs, rps[:, :])
                        nc.vector.tensor_tensor(
                            out=ctxT[pb:pb + 64, hp, ksl],
                            in0=pv[0:64, :], in1=rrs, op=OP.mult)
                    # ---- out-proj partial for this k-half + ReduceScatter;
                    # psums borrow the freed pv banks (same tag rings).
                    y_sb = ypool.tile([P, 4, D], FP8, tag="y")
                    rs_in = dpool.tile([512, D], FP8, tag=f"rsin{kb}")
                    for tb in range(4):
                        tsl = slice(kb * 512 + tb * P, kb * 512 + (tb + 1) * P)
                        for db in range(D // 512):
                            dsl = slice(db * 512, (db + 1) * 512)
                            ps = psPV.tile([P, 512], F32,
                                           tag=f"pv{(tb * 2 + db) % 4}",
                                           name=f"op_{kb}_{tb}_{db}")
                            for cc in range(2):
                                nc.tensor.matmul(
                                    ps[:, :],
                                    ctxT[:, cc, tsl],
                                    wo_bf[:, cc, dsl],
                                    start=(cc == 0), stop=(cc == 1))
                            if last and db == 1:
                                nc.scalar.copy(y_sb[:, tb, dsl], ps[:, :])
                            else:
                                nc.vector.tensor_copy(y_sb[:, tb, dsl],
                                                      ps[:, :])
                        nc.gpsimd.dma_start(
                            out=rs_in[tb * P:(tb + 1) * P, :],
                            in_=y_sb[:, tb, :])
                    nc.gpsimd.collective_compute(
                        "ReduceScatter", OP.add, replica_groups=groups,
                        ins=[rs_in.opt()], outs=[rs_out[kb].ap().opt()])

            # ====== residual + LN_f + FFN per k-half (kt0 under RS1) ====
            # psFX (4 banks) lands in the freed score-ring zone so FFN1 can
            # start during the kb=1 epilogue; psH2 reuses the pv zone.
            with (
                tc.tile_pool(name="ffn", bufs=1) as fp,
                tc.tile_pool(name="fstream", bufs=2) as fs,
                tc.tile_pool(name="psFX", bufs=1, space="PSUM") as psFX,
                tc.tile_pool(name="psH2", bufs=1, space="PSUM") as psH2,
            ):
                h1T = fp.tile([P, D2 // P, KQ], BF16)
                o_sb = fp.tile([P, 2, D], F32)
                for kt in range(2):
                    rs_sb = fs.tile([P, D], FP8, tag="rs")
                    nc.sync.dma_start(out=rs_sb, in_=rs_out[kt].ap())
                    nc.vector.tensor_tensor(out=x_sb[:, kt, :], in0=rs_sb,
                                            in1=qres_sb[:, kt, :], op=OP.add)
                    mv2 = fs.tile([P, 1, 2], F32, tag="ln_mv")
                    _ln_stats(nc, fs, x_sb[:, kt, :], mv2[:, 0, :])
                    rs1c = _rsqrt_newton(nc, fs, mv2[:, 0, 1:2])
                    xn = _ln_norm(nc, fs, x_sb[:, kt, :], mv2[:, 0, 0:1],
                                  rs1c[:, 0:1])
                    tpx = psFX.tile([P, DC, P], BF16, tag="tpx")
                    for dc in range(DC):
                        nc.tensor.transpose(
                            tpx[:, dc, :], xn[:, dc * P:(dc + 1) * P], ident)
                    nc.scalar.copy(xfT[:, :, kt * P:(kt + 1) * P], tpx)
                    # FFN1 flipped: xfT chunks stationary, w1 streams
                    h1 = fs.tile([P, 4, 512], BF16, tag="h1")
                    for hb in range(4):
                        ps = psFX.tile([P, 512], F32, tag="f", bufs=3)
                        hsl = slice(hb * 512, (hb + 1) * 512)
                        for dc in range(DC):
                            nc.tensor.matmul(
                                ps[:, :], xfT[:, dc, kt * P:(kt + 1) * P],
                                w1_bf[:, dc, hsl],
                                start=(dc == 0), stop=False)
                        nc.tensor.matmul(
                            ps[:, :], ones_row[0:1, 0:P],
                            b1_bf[0:1, hsl], start=False, stop=True)
                        nc.scalar.activation(h1[:, hb, :], ps[:, :], AF.Gelu)
                    tph = psH2.tile([P, D2 // P, P], BF16, tag="tph")
                    h1f = h1[:, :, :].rearrange("p a b -> p (a b)")
                    for hc in range(D2 // P):
                        nc.tensor.transpose(
                            tph[:, hc, :], h1f[:, hc * P:(hc + 1) * P], ident)
                    nc.scalar.copy(h1T[:, 0:8, kt * P:(kt + 1) * P],
                                   tph[:, 0:8, :])
                    nc.vector.tensor_copy(h1T[:, 8:16, kt * P:(kt + 1) * P],
                                          tph[:, 8:16, :])
                    # FFN2: accumulate over hc chunks, both D halves live
                    ps0 = psH2.tile([P, 512], F32, tag="o0")
                    ps1 = psH2.tile([P, 512], F32, tag="o1")
                    for hc in range(D2 // P):
                        for db, ps in ((0, ps0), (1, ps1)):
                            nc.tensor.matmul(
                                ps[:, :], h1T[:, hc, kt * P:(kt + 1) * P],
                                w2_bf[:, hc, db * 512:(db + 1) * 512],
                                start=(hc == 0), stop=False)
                    for db, ps in ((0, ps0), (1, ps1)):
                        dsl = slice(db * 512, (db + 1) * 512)
                        nc.tensor.matmul(
                            ps[:, :], ones_row[0:1, 0:P],
                            b2_bf[0:1, dsl], start=False, stop=True)
                        nc.vector.tensor_tensor(out=o_sb[:, kt, dsl],
                                                in0=ps[:, :],
                                                in1=x_sb[:, kt, dsl],
                                                op=OP.add)
                        nc.sync.dma_start(
                            out=out_d.ap()[kt * P:(kt + 1) * P, dsl],
                            in_=o_sb[:, kt, dsl])
            wt_cm.__exit__(None, None, None)

    nc.compile()
    return nc


def _get_nc():
    if "nc" not in _CACHE:
        _CACHE["nc"] = _build_nc()
    return _CACHE["nc"]


def _softplus(x):
    return float(np.log1p(np.exp(np.float64(x))))


def kernel(**inputs):
    f = lambda name: np.ascontiguousarray(np.asarray(inputs[name], np.float32))
    q = f("q"); kv = f("kv"); ab = f("attn_bias"); ob = f("obs_bias")
    density = f("density")
    c1 = _softplus(inputs["dist_raw"])
    c2 = _softplus(inputs["obs_raw"])
    tg = float(np.tanh(np.float64(np.asarray(inputs["dens_raw"], np.float64))))
    gate = (1.0 + tg * density).astype(np.float32)       # [B, K]

    ln_q_w = f("ln_q_w"); ln_q_b = f("ln_q_b")
    ln_kv_w = f("ln_kv_w"); ln_kv_b = f("ln_kv_b")
    ln_f_w = f("ln_f_w"); ln_f_b = f("ln_f_b")
    scale = np.float32(HD ** -0.5)
    wq = scale * ln_q_w[:, None] * f("wq")
    bq = scale * (ln_q_b @ f("wq") + f("bq"))
    wk = ln_kv_w[:, None] * f("wk"); bk = ln_kv_b @ f("wk") + f("bk")
    wv = ln_kv_w[:, None] * f("wv"); bv = ln_kv_b @ f("wv") + f("bv")
    w1 = ln_f_w[:, None] * f("w1"); b1 = ln_f_b @ f("w1") + f("b1")
    wo = f("wo"); bo = f("bo"); w2 = f("w2"); b2 = f("b2")

    # host-side: exp of the gated bias sum, transposed to [S, K] bf16
    et_host = []
    for b in range(B):
        cb = (c1 * ab[b] + c2 * ob[b]) * gate[b][:, None]   # [K, S]
        et_host.append(np.ascontiguousarray(
            np.exp(cb.T).astype(ml_dtypes.bfloat16)))        # [S, K]

    cont = np.ascontiguousarray
    bf = lambda a: np.ascontiguousarray(np.asarray(a, dtype=ml_dtypes.bfloat16))
    in_maps = []
    row_maps = []
    for c in range(N_CORES):
        b, r = divmod(c, 4)
        hs = slice(r * DH, (r + 1) * DH)
        rows = np.r_[r * P:(r + 1) * P, 512 + r * P:512 + (r + 1) * P]
        row_maps.append((b, rows))
        in_maps.append({
            "q": bf(q[b]), "kv": bf(kv[b]),
            "et": et_host[b],
            "gate": cont(gate[b][:, None]),
            "growb": bf(gate[b][None, :]),
            "q_res": cont(q[b][rows] + bo[None, :]),
            "wq": bf(wq[:, hs]), "wk": bf(wk[:, hs]), "wv": bf(wv[:, hs]),
            "bq": bf(bq[None, hs]), "bk": cont(bk[hs, None]),
            "bv": cont(bv[hs, None]),
            "wo": bf(wo[hs, :]), "w1": bf(w1), "b1": bf(b1[None, :]),
            "w2": bf(w2), "b2": bf(b2[None, :]),
        })

    global _last_in_maps
    _last_in_maps = in_maps
    nc = _get_nc()
    res = bass_utils.run_bass_kernel_spmd(
        nc, in_maps, core_ids=list(range(N_CORES)))
    out = np.empty((B, K, D), np.float32)
    for c in range(N_CORES):
        b, rows = row_maps[c]
        out[b][rows] = res.results[c]["xq"]
    return out


# revision 34
# speedup vs baseline: 1.0205x; 1.0205x over previous
"""CrossAttnBlock on 8 trn2 NeuronCores.

Sharding: core c -> batch b=c//4, rank r=c%4 within the batch group.
Attention is Megatron-sliced over heads (4 of 16 per core); the
out-projection partial sums are combined with TWO chunked bf16
ReduceScatters (one per K-half), each launched as soon as its half of
the attention context is ready.  After the RS, core r owns k-rows
{r*128..+128} of each half; the host maps them back.

Key optimizations vs the 636us baseline:
- ALL transposes (q, kv, v-ctx, FFN x / h1) moved from DMA-transpose
  (5.7us each, serialized on one ring) to PE-array transposes (53ns per
  128x128 chunk) + one PSUM->SBUF copy per tile.  The PE p-state ramp
  (0.65/1.2/2.4 GHz) rewards a continuously-busy PE; the old prologue
  idled the PE for 300us waiting on the sync DMA ring.
- q/kv stream in as bf16 (halves input DMA; LN stats lose nothing vs
  the 2e-2 tolerance) and the LN normalize hits the DVE 4x mode.
- LN inv-std batched over tile PAIRS (the [128,1] ACT ops have a
  ~350-cycle fixed overhead) using Sqrt+DVE-reciprocal: Sqrt shares an
  ACT table set with the Identity/Copy drains, so the whole prologue
  takes ONE table load (Ln/Exp alternation was 40 loads = 51us).  The
  FFN LNs use a 3-step Newton rsqrt on the DVE (no ACT tables at all).
- The previous s-group's projection chains are emitted BETWEEN the next
  group's LN/transpose pairs so the PE never starves in the prologue.
- ReduceScatter payload in fp8e4 (partial sums ~N(0,0.5); the wire
  phase drops ~2us and staging DMA halves).
- exp(scores) stays on the scalar/ACT engine (the only table engine):
  128 x [128,1024] = hard ~141us floor; everything else in the
  attention phase is kept OFF the scalar engine so exp back-to-backs.
- Attention epilogue (denominators + out-proj) borrows the freed pv
  PSUM banks (per-tag rings) instead of the score ring, so the next
  K-half's QK->exp pipeline restarts immediately.
- DMA rings: Pool ring = bulk loads + rs_in staging + collectives (the
  rs_in stores precede their collective in ring order), SP ring = et
  stream + rs_out loads + output stores.  The ACT ring stays empty so
  exp dispatch is never stalled; q/kv loads stay off the SP ring, which
  doubles as the semaphore hub.
"""
import sys
import numpy as np

sys.path.insert(0, "/opt/trn_rl_repo")

import ml_dtypes  # noqa: E402
import concourse.bass as bass  # noqa: E402
import concourse.mybir as mybir  # noqa: E402
import concourse.tile as tile  # noqa: E402
from concourse import bacc  # noqa: E402
from concourse import bass_utils  # noqa: E402
from concourse.masks import make_identity  # noqa: E402

F32 = mybir.dt.float32
BF16 = mybir.dt.bfloat16
FP8 = mybir.dt.float8e4
AF = mybir.ActivationFunctionType
OP = mybir.AluOpType

D = 1024
H = 16
HD = 64
B = 2
K = 1024
S = 4096
EPS = 1e-5
N_CORES = 8
KQ = K // 4          # rows per core after the two ReduceScatters
HC = 4               # heads per core
DH = HC * HD         # ctx dims per core (256)
P = 128
DC = D // P          # 8 D-chunks
D2 = 2 * D

_CACHE = {}


def _ln_stats(nc, pool, xt, mv, eng=None):
    """bn stats of xt [128, 1024] into mv [128, 2] (mean, var)."""
    if eng is None:
        eng = nc.vector
    st = pool.tile([P, 2, 6], F32, tag="ln_st")
    xs = xt.rearrange("p (s f) -> p s f", s=2)
    for i in range(2):
        eng.bn_stats(out=st[:, i, :], in_=xs[:, i, :])
    eng.bn_aggr(out=mv, in_=st[:, :, :])


def _ln_norm(nc, pool, xt, mean_col, rs_col):
    xn = pool.tile([P, D], BF16, tag="ln_out")
    nc.vector.tensor_scalar(out=xn, in0=xt, scalar1=mean_col, scalar2=rs_col,
                            op0=OP.subtract, op1=OP.mult)
    return xn


def _inv_std(nc, pool, mv2, n, gate_cols=None):
    """inv-std for n tiles batched: mv2 [128, n, 2] -> rs [128, n].
    Sqrt on ACT (shares its table set with the Identity/Copy drains, so
    the whole prologue needs ONE table load) + reciprocal on DVE."""
    sq = pool.tile([P, n], F32, tag="ln_sq")
    nc.scalar.activation(sq, mv2[:, :, 1], AF.Sqrt, bias=nc._eps_t[:, :],
                         scale=1.0)
    rs = pool.tile([P, n], F32, tag="ln_rs")
    nc.vector.reciprocal(rs, sq)
    if gate_cols is not None:
        nc.vector.tensor_tensor(out=rs, in0=rs, in1=gate_cols, op=OP.mult)
    return rs


def _rsqrt_newton(nc, pool, v_col, n=1):
    """1/sqrt(v+eps) on DVE only (no ACT tables): seed y0=1/v, then 3
    Newton steps y <- y*(1.5 - 0.5*v*y^2).  Post-residual LN variance
    sits in ~[0.8, 1.6], where this converges to ~1e-4."""
    v = pool.tile([P, n], F32, tag="nw_v")
    nc.vector.tensor_scalar_add(v, v_col, nc._eps_t[:, 0:1])
    y = pool.tile([P, n], F32, tag="nw_y")
    nc.vector.reciprocal(y, v)
    hv = pool.tile([P, n], F32, tag="nw_hv")
    nc.vector.tensor_scalar_mul(hv, v, -0.5)
    for _ in range(3):
        y2 = pool.tile([P, n], F32, tag="nw_y2", bufs=2)
        nc.vector.tensor_tensor(out=y2, in0=y, in1=y, op=OP.mult)
        t = pool.tile([P, n], F32, tag="nw_t", bufs=2)
        nc.vector.tensor_scalar(out=t, in0=y2, scalar1=hv[:, 0:1],
                                scalar2=1.5, op0=OP.mult, op1=OP.add)
        yn = pool.tile([P, n], F32, tag="nw_yn", bufs=2)
        nc.vector.tensor_tensor(out=yn, in0=y, in1=t, op=OP.mult)
        y = yn
    return y


def _build_nc():
    nc = bacc.Bacc("TRN2", target_bir_lowering=False, debug=False,
                   num_devices=N_CORES)

    def din(name, shape, dt=F32):
        return nc.dram_tensor(name, shape, dt, kind="ExternalInput")

    q_d = din("q", [K, D], BF16)
    kv_d = din("kv", [S, D], BF16)
    et_d = din("et", [S, K], BF16)
    gate_d = din("gate", [K, 1])
    qres_d = din("q_res", [KQ, D])
    wq_d = din("wq", [D, DH], BF16)
    wk_d = din("wk", [D, DH], BF16)
    wv_d = din("wv", [D, DH], BF16)
    bq_d = din("bq", [1, DH], BF16)
    bk_d = din("bk", [DH, 1])
    bv_d = din("bv", [DH, 1])
    grow_d = din("growb", [1, K], BF16)
    wo_d = din("wo", [DH, D], BF16)
    w1_d = din("w1", [D, D2], BF16)
    b1_d = din("b1", [1, D2], BF16)
    w2_d = din("w2", [D2, D], BF16)
    b2_d = din("b2", [1, D], BF16)
    out_d = nc.dram_tensor("xq", [KQ, D], F32, kind="ExternalOutput")

    rs_out = [nc.dram_tensor(f"rs_out{i}", [P, D], FP8) for i in range(2)]
    groups = [[0, 1, 2, 3], [4, 5, 6, 7]]

    with tile.TileContext(nc) as tc:
        with (
            tc.tile_pool(name="const", bufs=1) as cpool,
            tc.tile_pool(name="persist", bufs=1) as pp,
            tc.tile_pool(name="dram", bufs=1, space="DRAM") as dpool,
        ):
            # ---- constants ----
            eps_t = cpool.tile([P, 1], F32)
            nc.vector.memset(eps_t, EPS)
            nc._eps_t = eps_t
            ident = cpool.tile([P, P], BF16)
            make_identity(nc, ident)
            ones_row = cpool.tile([1, 512], BF16)
            nc.vector.memset(ones_row, 1.0)
            ones64 = cpool.tile([P, 64], BF16)
            nc.vector.memset(ones64, 1.0)
            gsb = cpool.tile([P, DC], F32)
            nc.sync.dma_start(out=gsb, in_=gate_d.ap().rearrange(
                "(t p) o -> p (t o)", p=P))
            grow_bf = cpool.tile([1, K], BF16)
            nc.sync.dma_start(out=grow_bf, in_=grow_d[:, :])
            bq_bf = cpool.tile([1, DH], BF16)
            nc.sync.dma_start(out=bq_bf, in_=bq_d[:, :])
            bk_col = cpool.tile([P, 2], F32)
            nc.sync.dma_start(out=bk_col, in_=bk_d.ap().rearrange(
                "(c p) o -> p (c o)", p=P))
            bv_col = cpool.tile([P, 2], F32)
            nc.sync.dma_start(out=bv_col, in_=bv_d.ap().rearrange(
                "(c p) o -> p (c o)", p=P))
            b1_bf = cpool.tile([1, D2], BF16)
            nc.sync.dma_start(out=b1_bf, in_=b1_d[:, :])
            b2_bf = cpool.tile([1, D], BF16)
            nc.sync.dma_start(out=b2_bf, in_=b2_d[:, :])

            # ---- persistent activation tensors ----
            qpT = pp.tile([P, 2, K], BF16)       # [2 heads x 64, hp, k]
            kpT = pp.tile([P, 2, S], BF16)
            vp = pp.tile([P, 32, HC * 65], BF16)  # [s%128, s//128, h*65+(hd|one)]
            ctxT = pp.tile([P, 2, K], BF16)

            # ============ q: LN+gate -> PE transpose -> projection ========
            with (
                tc.tile_pool(name="projw", bufs=1) as wpool,
                tc.tile_pool(name="psA", bufs=4, space="PSUM") as psA,
                tc.tile_pool(name="psT", bufs=3, space="PSUM") as psT,
            ):
                wq_bf = wpool.tile([P, DC, DH], BF16)
                nc.gpsimd.dma_start(out=wq_bf, in_=wq_d.ap().rearrange(
                    "(c p) n -> p c n", p=P))
                wk_bf = wpool.tile([P, DC, DH], BF16)
                nc.gpsimd.dma_start(out=wk_bf, in_=wk_d.ap().rearrange(
                    "(c p) n -> p c n", p=P))
                wv_bf = wpool.tile([P, DC, DH], BF16)
                nc.gpsimd.dma_start(out=wv_bf, in_=wv_d.ap().rearrange(
                    "(c p) n -> p c n", p=P))

                def ln_pair(lpool, xts, gate_cols=None):
                    """LN a pair of loaded [128, D] tiles -> two bf16 tiles."""
                    mv2 = lpool.tile([P, 2, 2], F32, tag="ln_mv")
                    for i, xt in enumerate(xts):
                        _ln_stats(nc, lpool, xt, mv2[:, i, :])
                    rs2 = _inv_std(nc, lpool, mv2, 2, gate_cols)
                    return [
                        _ln_norm(nc, lpool, xt, mv2[:, i, 0:1], rs2[:, i:i + 1])
                        for i, xt in enumerate(xts)]

                def transp_tile(xn, dst, copy_eng):
                    tp = psT.tile([P, DC, P], BF16, tag="tp")
                    for dc in range(DC):
                        nc.tensor.transpose(
                            tp[:, dc, :], xn[:, dc * P:(dc + 1) * P], ident)
                    if copy_eng == "s":
                        nc.scalar.copy(dst, tp)
                    else:
                        nc.vector.tensor_copy(dst, tp)

                def q_proj_unit(hp, tb, qT):
                    # q projection chain: psum[2hd, 512 tok] (+rank-1 gate
                    # bias term)
                    ps = psA.tile([P, 512], F32, tag="mm")
                    for dc in range(DC):
                        nc.tensor.matmul(
                            ps[:, :],
                            wq_bf[:, dc, hp * P:(hp + 1) * P],
                            qT[:, dc, tb * 512:(tb + 1) * 512],
                            start=(dc == 0), stop=False)
                    nc.tensor.matmul(
                        ps[:, :], bq_bf[0:1, hp * P:(hp + 1) * P],
                        grow_bf[0:1, tb * 512:(tb + 1) * 512],
                        start=False, stop=True)
                    nc.scalar.activation(
                        qpT[:, hp, tb * 512:(tb + 1) * 512], ps[:, :],
                        AF.Identity)

                def kv_proj_unit(sg, hp, sb_, which, kvT, vpT):
                    # one k- or v-projection chain (bias folded into drain)
                    ssl = slice(sb_ * 512, (sb_ + 1) * 512)
                    w_bf = wk_bf if which == "k" else wv_bf
                    ps = psA.tile([P, 512], F32, tag="mm")
                    for dc in range(DC):
                        nc.tensor.matmul(
                            ps[:, :],
                            w_bf[:, dc, hp * P:(hp + 1) * P],
                            kvT[:, dc, ssl],
                            start=(dc == 0), stop=(dc == DC - 1))
                    if which == "k":
                        osl = slice(sg * 1024 + sb_ * 512,
                                    sg * 1024 + (sb_ + 1) * 512)
                        nc.scalar.activation(
                            kpT[:, hp, osl], ps[:, :], AF.Identity,
                            bias=bk_col[:, hp:hp + 1], scale=1.0)
                    else:
                        nc.scalar.activation(
                            vpT[:, hp, ssl], ps[:, :], AF.Identity,
                            bias=bv_col[:, hp:hp + 1], scale=1.0)

                def vp_unit(sg, hp, vpT):
                    # vp natural layout [s, hd] per head (+ ones column):
                    # PE-transpose vpT chunks, then strided inserts.
                    tp = psT.tile([P, DC, P], BF16, tag="tp")
                    for dc in range(DC):
                        nc.tensor.transpose(
                            tp[:, dc, :],
                            vpT[:, hp, dc * P:(dc + 1) * P], ident)
                    for half in range(2):
                        h = hp * 2 + half
                        if half == 0:
                            nc.vector.tensor_copy(
                                vp[:, sg * 8:(sg + 1) * 8,
                                   h * 65:h * 65 + 64],
                                tp[:, :, half * 64:half * 64 + 64])
                        else:
                            nc.scalar.copy(
                                vp[:, sg * 8:(sg + 1) * 8,
                                   h * 65:h * 65 + 64],
                                tp[:, :, half * 64:half * 64 + 64])

                # q LN/transposes, then kv by s-group.  The previous
                # group's projection chains are emitted BETWEEN the next
                # group's LN/transpose pairs so the PE never starves (and
                # stays at full p-state) while the DVE runs LN stats.
                qT = wpool.tile([P, DC, K], BF16, tag="qT")
                with (
                    tc.tile_pool(name="lnq", bufs=4) as lpool,
                    tc.tile_pool(name="lnkv_big", bufs=2) as kbig,
                ):
                    for tp_ in range(4):
                        xts = []
                        for i in range(2):
                            t = tp_ * 2 + i
                            qt = lpool.tile([P, D], BF16, tag=f"ln_in{i}")
                            nc.gpsimd.dma_start(
                                out=qt, in_=q_d[t * P:(t + 1) * P, :])
                            xts.append(qt)
                        qns = ln_pair(lpool, xts,
                                      gate_cols=gsb[:, tp_ * 2:tp_ * 2 + 2])
                        for i in range(2):
                            t = tp_ * 2 + i
                            transp_tile(qns[i], qT[:, :, t * P:(t + 1) * P],
                                        "s")
                    for h in range(HC):
                        nc.vector.memset(vp[:, :, h * 65 + 64:h * 65 + 65], 1.0)
                    pending = [lambda hp=hp, tb=tb: q_proj_unit(hp, tb, qT)
                               for hp in range(2) for tb in range(2)]
                    for sg in range(4):
                        kvT = kbig.tile([P, DC, 1024], BF16, tag="kvT")
                        vpT = kbig.tile([P, 2, 1024], BF16, tag="vpT")
                        for tp_ in range(4):
                            xts = []
                            for i in range(2):
                                t = tp_ * 2 + i
                                st_ = sg * 1024 + t * P
                                xt = lpool.tile([P, D], BF16, tag=f"ln_in{i}")
                                nc.gpsimd.dma_start(
                                    out=xt, in_=kv_d[st_:st_ + P, :])
                                xts.append(xt)
                            xns = ln_pair(lpool, xts)
                            for i in range(2):
                                t = tp_ * 2 + i
                                transp_tile(
                                    xns[i], kvT[:, :, t * P:(t + 1) * P],
                                    "s")
                            n_emit = (len(pending) + 3 - tp_) // (4 - tp_)
                            for u in pending[:n_emit]:
                                u()
                            pending = pending[n_emit:]
                        assert not pending
                        pending = []
                        for hp in range(2):
                            for sb_ in range(2):
                                for which in ("k", "v"):
                                    pending.append(
                                        lambda sg=sg, hp=hp, sb_=sb_,
                                        which=which, kvT=kvT, vpT=vpT:
                                        kv_proj_unit(sg, hp, sb_, which,
                                                     kvT, vpT))
                            pending.append(
                                lambda sg=sg, hp=hp, vpT=vpT:
                                vp_unit(sg, hp, vpT))
                        # order: k/v chains for hp then its vp transpose
                    for u in pending:
                        u()

            # tail weights: pool ring is idle once q/kv loads are done;
            # these loads overlap the start of attention.  The wt pool is
            # entered only now so its 76KB/partition comes from the freed
            # prologue zones instead of shrinking them.
            wt_cm = tc.tile_pool(name="wt", bufs=1)
            wt = wt_cm.__enter__()
            wo_bf = wt.tile([P, 2, D], BF16)
            nc.gpsimd.dma_start(out=wo_bf, in_=wo_d.ap().rearrange(
                "(c p) n -> p c n", p=P))
            w1_bf = wt.tile([P, DC, D2], BF16)
            nc.gpsimd.dma_start(out=w1_bf, in_=w1_d.ap().rearrange(
                "(c p) n -> p c n", p=P))
            w2_bf = wt.tile([P, D2 // P, D], BF16)
            nc.gpsimd.dma_start(out=w2_bf, in_=w2_d.ap().rearrange(
                "(c p) n -> p c n", p=P))
            qres_sb = wt.tile([P, 2, D], F32)
            nc.gpsimd.dma_start(out=qres_sb, in_=qres_d.ap().rearrange(
                "(t p) d -> p t d", p=P))
            x_sb = wt.tile([P, 2, D], F32)
            xfT = wt.tile([P, DC, KQ], BF16)

            # ======================= attention ==========================
            # loop kb (k halves) -> sc (s tiles) -> hp (head pairs);
            # the bias-exp block streams from HBM per (kb, sc).  After each
            # kb, the out-proj partial for that k-half is computed and its
            # ReduceScatter launched (overlapping the next kb / the FFN).
            with (
                tc.tile_pool(name="att", bufs=6) as apool,
                tc.tile_pool(name="ets", bufs=6) as espool,
                tc.tile_pool(name="attr", bufs=2) as rpool,
                tc.tile_pool(name="ysb", bufs=1) as ypool,
                tc.tile_pool(name="psS", bufs=2, space="PSUM") as psS,
                tc.tile_pool(name="psPV", bufs=1, space="PSUM") as psPV,
            ):
                def qk_exp_mult(kb, sc):
                    # scores -> exp -> bias-multiply for one s-tile; the
                    # returned `at` tiles live in the 6-deep ring until
                    # their PV matmuls consume them.
                    ksl = slice(kb * 512, (kb + 1) * 512)
                    et_blk = espool.tile([P, 512], BF16, tag="et")
                    nc.sync.dma_start(
                        out=et_blk,
                        in_=et_d.ap()[sc * P:(sc + 1) * P, ksl])
                    ats = []
                    for hp in range(2):
                        sps = psS.tile([P, 1024], F32, tag="sps")
                        nc.tensor.matmul(
                            sps[:, 0:512],
                            kpT[0:64, hp, sc * P:(sc + 1) * P],
                            qpT[0:64, hp, ksl],
                            start=True, stop=True, tile_position=(0, 0))
                        nc.tensor.matmul(
                            sps[:, 512:1024],
                            kpT[64:128, hp, sc * P:(sc + 1) * P],
                            qpT[64:128, hp, ksl],
                            start=True, stop=True, tile_position=(64, 0))
                        eq = apool.tile([P, 1024], BF16, tag="eq")
                        nc.scalar.activation(eq, sps[:, :], AF.Exp)
                        at = apool.tile([P, 1024], BF16, tag="at")
                        et_v = et_blk[:, :].rearrange(
                            "p (o f) -> p o f", o=1).broadcast_to(
                            [P, 2, 512])
                        nc.vector.tensor_tensor(
                            out=at[:, :].rearrange("p (o f) -> p o f", o=2),
                            in0=eq[:, :].rearrange("p (o f) -> p o f", o=2),
                            in1=et_v, op=OP.mult)
                        ats.append(at)
                    return ats

                pre_ats = None
                for kb in range(K // 512):
                    ksl = slice(kb * 512, (kb + 1) * 512)
                    pvs = [psPV.tile([65, 512], F32, tag=f"pv{h}",
                                     name=f"pv_{kb}_{h}")
                           for h in range(HC)]
                    for sc in range(S // P):
                        if pre_ats is not None and sc < len(pre_ats):
                            ats = pre_ats[sc]
                        else:
                            ats = qk_exp_mult(kb, sc)
                        for hp in range(2):
                            at = ats[hp]
                            he = hp * 2
                            ho = hp * 2 + 1
                            nc.tensor.matmul(
                                pvs[he][:, :],
                                vp[:, sc, he * 65:(he + 1) * 65],
                                at[:, 0:512],
                                start=(sc == 0), stop=(sc == S // P - 1))
                            nc.tensor.matmul(
                                pvs[ho][:, :],
                                vp[:, sc, ho * 65:(ho + 1) * 65],
                                at[:, 512:1024],
                                start=(sc == 0), stop=(sc == S // P - 1))
                    # prefetch the NEXT K-half's first scores/exp/multiply
                    # so the scalar engine keeps running through the
                    # epilogue below (its PE work sits ahead of the
                    # epilogue matmuls in the queue).
                    if kb == 0:
                        pre_ats = [qk_exp_mult(1, s) for s in range(2)]
                    # denominators: spread the 4 heads onto partitions
                    # {0,32,64,96} so one reciprocal covers all of them;
                    # broadcast 1/den across 64 partitions with a 1-row PE
                    # matmul whose psum borrows a freed score-ring slot.
                    last = kb == K // 512 - 1
                    dall = rpool.tile([97, 512], F32, tag="dall")
                    for h in range(HC):
                        if h % 2 == 1:
                            nc.scalar.copy(dall[32 * h:32 * h + 1, :],
                                           pvs[h][64:65, :])
                        else:
                            nc.vector.tensor_copy(dall[32 * h:32 * h + 1, :],
                                                  pvs[h][64:65, :])
                    rden = rpool.tile([97, 512], BF16, tag="rden")
                    with nc.allow_low_precision(
                            reason="softmax denom reciprocal in bf16; "
                                   "ctx is bf16 anyway"):
                        nc.vector.reciprocal(rden, dall[:, :])
                    for h in range(HC):
                        pv = pvs[h]
                        pb = (h % 2) * 64
                        hp = h // 2
                        rps = psS.tile([64, 512], F32, tag="sps",
                                       name=f"rps_{kb}_{h}")
                        nc.tensor.matmul(rps[:, :],
                                         ones64[32 * h:32 * h + 1, :],
                                         rden[32 * h:32 * h + 1, :],
                                         start=True, stop=True,
                                         tile_position=(32 * h, 0))
                        rrs = rpool.tile([64, 512], BF16, tag="rrs")
                        if last and h % 2 == 1:
                            nc.scalar.copy(rrs, rps[:, :])
                        else:
                            nc.vector.tensor_copy(rrs, rps[:, :])
                        nc.vector.tensor_tensor(
                            out=ctxT[pb:pb + 64, hp, ksl],
                            in0=pv[0:64, :], in1=rrs, op=OP.mult)
                    # ---- out-proj partial for this k-half + ReduceScatter;
                    # psums borrow the freed pv banks (same tag rings).
                    y_sb = ypool.tile([P, 4, D], FP8, tag="y")
                    rs_in = dpool.tile([512, D], FP8, tag=f"rsin{kb}")
                    for tb in range(4):
                        tsl = slice(kb * 512 + tb * P, kb * 512 + (tb + 1) * P)
                        for db in range(D // 512):
                            dsl = slice(db * 512, (db + 1) * 512)
                            ps = psPV.tile([P, 512], F32,
                                           tag=f"pv{(tb * 2 + db) % 4}",
                                           name=f"op_{kb}_{tb}_{db}")
                            for cc in range(2):
                                nc.tensor.matmul(
                                    ps[:, :],
                                    ctxT[:, cc, tsl],
                                    wo_bf[:, cc, dsl],
                                    start=(cc == 0), stop=(cc == 1))
                            if last and db == 1:
                                nc.scalar.copy(y_sb[:, tb, dsl], ps[:, :])
                            else:
                                nc.vector.tensor_copy(y_sb[:, tb, dsl],
                                                      ps[:, :])
                        nc.gpsimd.dma_start(
                            out=rs_in[tb * P:(tb + 1) * P, :],
                            in_=y_sb[:, tb, :])
                    nc.gpsimd.collective_compute(
                        "ReduceScatter", OP.add, replica_groups=groups,
                        ins=[rs_in.opt()], outs=[rs_out[kb].ap().opt()])

            # ====== residual + LN_f + FFN per k-half (kt0 under RS1) ====
            # psFX (4 banks) lands in the freed score-ring zone so FFN1 can
            # start during the kb=1 epilogue; psH2 reuses the pv zone.
            with (
                tc.tile_pool(name="ffn", bufs=1) as fp,
                tc.tile_pool(name="fstream", bufs=2) as fs,
                tc.tile_pool(name="psFX", bufs=1, space="PSUM") as psFX,
                tc.tile_pool(name="psH2", bufs=1, space="PSUM") as psH2,
            ):
                h1T = fp.tile([P, D2 // P, KQ], BF16)
                o_sb = fp.tile([P, 2, D], F32)
                for kt in range(2):
                    rs_sb = fs.tile([P, D], FP8, tag="rs")
                    nc.sync.dma_start(out=rs_sb, in_=rs_out[kt].ap())
                    nc.vector.tensor_tensor(out=x_sb[:, kt, :], in0=rs_sb,
                                            in1=qres_sb[:, kt, :], op=OP.add)
                    mv2 = fs.tile([P, 1, 2], F32, tag="ln_mv")
                    _ln_stats(nc, fs, x_sb[:, kt, :], mv2[:, 0, :])
                    rs1c = _rsqrt_newton(nc, fs, mv2[:, 0, 1:2])
                    xn = _ln_norm(nc, fs, x_sb[:, kt, :], mv2[:, 0, 0:1],
                                  rs1c[:, 0:1])
                    tpx = psFX.tile([P, DC, P], BF16, tag="tpx")
                    for dc in range(DC):
                        nc.tensor.transpose(
                            tpx[:, dc, :], xn[:, dc * P:(dc + 1) * P], ident)
                    nc.scalar.copy(xfT[:, :, kt * P:(kt + 1) * P], tpx)
                    # FFN1 flipped: xfT chunks stationary, w1 streams
                    h1 = fs.tile([P, 4, 512], BF16, tag="h1")
                    for hb in range(4):
                        ps = psFX.tile([P, 512], F32, tag="f", bufs=3)
                        hsl = slice(hb * 512, (hb + 1) * 512)
                        for dc in range(DC):
                            nc.tensor.matmul(
                                ps[:, :], xfT[:, dc, kt * P:(kt + 1) * P],
                                w1_bf[:, dc, hsl],
                                start=(dc == 0), stop=False)
                        nc.tensor.matmul(
                            ps[:, :], ones_row[0:1, 0:P],
                            b1_bf[0:1, hsl], start=False, stop=True)
                        nc.scalar.activation(h1[:, hb, :], ps[:, :], AF.Gelu)
                    tph = psH2.tile([P, D2 // P, P], BF16, tag="tph")
                    h1f = h1[:, :, :].rearrange("p a b -> p (a b)")
                    for hc in range(D2 // P):
                        nc.tensor.transpose(
                            tph[:, hc, :], h1f[:, hc * P:(hc + 1) * P], ident)
                    nc.scalar.copy(h1T[:, 0:8, kt * P:(kt + 1) * P],
                                   tph[:, 0:8, :])
                    nc.vector.tensor_copy(h1T[:, 8:16, kt * P:(kt + 1) * P],
                                          tph[:, 8:16, :])
                    # FFN2: accumulate over hc chunks, both D halves live
                    ps0 = psH2.tile([P, 512], F32, tag="o0")
                    ps1 = psH2.tile([P, 512], F32, tag="o1")
                    for hc in range(D2 // P):
                        for db, ps in ((0, ps0), (1, ps1)):
                            nc.tensor.matmul(
                                ps[:, :], h1T[:, hc, kt * P:(kt + 1) * P],
                                w2_bf[:, hc, db * 512:(db + 1) * 512],
                                start=(hc == 0), stop=False)
                    for db, ps in ((0, ps0), (1, ps1)):
                        dsl = slice(db * 512, (db + 1) * 512)
                        nc.tensor.matmul(
                            ps[:, :], ones_row[0:1, 0:P],
                            b2_bf[0:1, dsl], start=False, stop=True)
                        nc.vector.tensor_tensor(out=o_sb[:, kt, dsl],
                                                in0=ps[:, :],
                                                in1=x_sb[:, kt, dsl],
                                                op=OP.add)
                        nc.sync.dma_start(
                            out=out_d.ap()[kt * P:(kt + 1) * P, dsl],
                            in_=o_sb[:, kt, dsl])
            wt_cm.__exit__(None, None, None)

    nc.compile()
    return nc


def _get_nc():
    if "nc" not in _CACHE:
        _CACHE["nc"] = _build_nc()
    return _CACHE["nc"]


def _softplus(x):
    return float(np.log1p(np.exp(np.float64(x))))


def kernel(**inputs):
    f = lambda name: np.ascontiguousarray(np.asarray(inputs[name], np.float32))
    q = f("q"); kv = f("kv"); ab = f("attn_bias"); ob = f("obs_bias")
    density = f("density")
    c1 = _softplus(inputs["dist_raw"])
    c2 = _softplus(inputs["obs_raw"])
    tg = float(np.tanh(np.float64(np.asarray(inputs["dens_raw"], np.float64))))
    gate = (1.0 + tg * density).astype(np.float32)       # [B, K]

    ln_q_w = f("ln_q_w"); ln_q_b = f("ln_q_b")
    ln_kv_w = f("ln_kv_w"); ln_kv_b = f("ln_kv_b")
    ln_f_w = f("ln_f_w"); ln_f_b = f("ln_f_b")
    scale = np.float32(HD ** -0.5)
    wq = scale * ln_q_w[:, None] * f("wq")
    bq = scale * (ln_q_b @ f("wq") + f("bq"))
    wk = ln_kv_w[:, None] * f("wk"); bk = ln_kv_b @ f("wk") + f("bk")
    wv = ln_kv_w[:, None] * f("wv"); bv = ln_kv_b @ f("wv") + f("bv")
    w1 = ln_f_w[:, None] * f("w1"); b1 = ln_f_b @ f("w1") + f("b1")
    wo = f("wo"); bo = f("bo"); w2 = f("w2"); b2 = f("b2")

    # host-side: exp of the gated bias sum, transposed to [S, K] bf16
    et_host = []
    for b in range(B):
        cb = (c1 * ab[b] + c2 * ob[b]) * gate[b][:, None]   # [K, S]
        et_host.append(np.ascontiguousarray(
            np.exp(cb.T).astype(ml_dtypes.bfloat16)))        # [S, K]

    cont = np.ascontiguousarray
    bf = lambda a: np.ascontiguousarray(np.asarray(a, dtype=ml_dtypes.bfloat16))
    in_maps = []
    row_maps = []
    for c in range(N_CORES):
        b, r = divmod(c, 4)
        hs = slice(r * DH, (r + 1) * DH)
        rows = np.r_[r * P:(r + 1) * P, 512 + r * P:512 + (r + 1) * P]
        row_maps.append((b, rows))
        in_maps.append({
            "q": bf(q[b]), "kv": bf(kv[b]),
            "et": et_host[b],
            "gate": cont(gate[b][:, None]),
            "growb": bf(gate[b][None, :]),
            "q_res": cont(q[b][rows] + bo[None, :]),
            "wq": bf(wq[:, hs]), "wk": bf(wk[:, hs]), "wv": bf(wv[:, hs]),
            "bq": bf(bq[None, hs]), "bk": cont(bk[hs, None]),
            "bv": cont(bv[hs, None]),
            "wo": bf(wo[hs, :]), "w1": bf(w1), "b1": bf(b1[None, :]),
            "w2": bf(w2), "b2": bf(b2[None, :]),
        })

    global _last_in_maps
    _last_in_maps = in_maps
    nc = _get_nc()
    res = bass_utils.run_bass_kernel_spmd(
        nc, in_maps, core_ids=list(range(N_CORES)))
    out = np.empty((B, K, D), np.float32)
    for c in range(N_CORES):
        b, rows = row_maps[c]
        out[b][rows] = res.results[c]["xq"]
    return out


# revision 35
# speedup vs baseline: 1.0482x; 1.0271x over previous
"""CrossAttnBlock on 8 trn2 NeuronCores.

Sharding: core c -> batch b=c//4, rank r=c%4 within the batch group.
Attention is Megatron-sliced over heads (4 of 16 per core); the
out-projection partial sums are combined with TWO chunked bf16
ReduceScatters (one per K-half), each launched as soon as its half of
the attention context is ready.  After the RS, core r owns k-rows
{r*128..+128} of each half; the host maps them back.

Key optimizations vs the 636us baseline:
- ALL transposes (q, kv, v-ctx, FFN x / h1) moved from DMA-transpose
  (5.7us each, serialized on one ring) to PE-array transposes (53ns per
  128x128 chunk) + one PSUM->SBUF copy per tile.  The PE p-state ramp
  (0.65/1.2/2.4 GHz) rewards a continuously-busy PE; the old prologue
  idled the PE for 300us waiting on the sync DMA ring.
- q/kv stream in as bf16 (halves input DMA; LN stats lose nothing vs
  the 2e-2 tolerance) and the LN normalize hits the DVE 4x mode.
- LN inv-std batched over tile PAIRS (the [128,1] ACT ops have a
  ~350-cycle fixed overhead) using Sqrt+DVE-reciprocal: Sqrt shares an
  ACT table set with the Identity/Copy drains, so the whole prologue
  takes ONE table load (Ln/Exp alternation was 40 loads = 51us).  The
  FFN LNs use a 3-step Newton rsqrt on the DVE (no ACT tables at all).
- The previous s-group's projection chains are emitted BETWEEN the next
  group's LN/transpose pairs so the PE never starves in the prologue.
- ReduceScatter payload in fp8e4 (partial sums ~N(0,0.5); the wire
  phase drops ~2us and staging DMA halves).
- exp(scores) stays on the scalar/ACT engine (the only table engine):
  128 x [128,1024] = hard ~141us floor; everything else in the
  attention phase is kept OFF the scalar engine so exp back-to-backs.
- Attention epilogue (denominators + out-proj) borrows the freed pv
  PSUM banks (per-tag rings) instead of the score ring, so the next
  K-half's QK->exp pipeline restarts immediately.
- DMA rings: Pool ring = bulk loads + rs_in staging + collectives (the
  rs_in stores precede their collective in ring order), SP ring = et
  stream + rs_out loads + output stores.  The ACT ring stays empty so
  exp dispatch is never stalled; q/kv loads stay off the SP ring, which
  doubles as the semaphore hub.
"""
import sys
import numpy as np

sys.path.insert(0, "/opt/trn_rl_repo")

import ml_dtypes  # noqa: E402
import concourse.bass as bass  # noqa: E402
import concourse.mybir as mybir  # noqa: E402
import concourse.tile as tile  # noqa: E402
from concourse import bacc  # noqa: E402
from concourse import bass_utils  # noqa: E402
from concourse.masks import make_identity  # noqa: E402

F32 = mybir.dt.float32
BF16 = mybir.dt.bfloat16
FP8 = mybir.dt.float8e4
AF = mybir.ActivationFunctionType
OP = mybir.AluOpType

D = 1024
H = 16
HD = 64
B = 2
K = 1024
S = 4096
EPS = 1e-5
N_CORES = 8
KQ = K // 4          # rows per core after the two ReduceScatters
HC = 4               # heads per core
DH = HC * HD         # ctx dims per core (256)
P = 128
DC = D // P          # 8 D-chunks
D2 = 2 * D

_CACHE = {}


def _ln_stats(nc, pool, xt, mv, eng=None):
    """bn stats of xt [128, 1024] into mv [128, 2] (mean, var)."""
    if eng is None:
        eng = nc.vector
    st = pool.tile([P, 2, 6], F32, tag="ln_st")
    xs = xt.rearrange("p (s f) -> p s f", s=2)
    for i in range(2):
        eng.bn_stats(out=st[:, i, :], in_=xs[:, i, :])
    eng.bn_aggr(out=mv, in_=st[:, :, :])


def _ln_norm(nc, pool, xt, mean_col, rs_col):
    xn = pool.tile([P, D], BF16, tag="ln_out")
    nc.vector.tensor_scalar(out=xn, in0=xt, scalar1=mean_col, scalar2=rs_col,
                            op0=OP.subtract, op1=OP.mult)
    return xn


def _inv_std(nc, pool, mv2, n, gate_cols=None):
    """inv-std for n tiles batched: mv2 [128, n, 2] -> rs [128, n].
    Sqrt on ACT (shares its table set with the Identity/Copy drains, so
    the whole prologue needs ONE table load) + reciprocal on DVE."""
    sq = pool.tile([P, n], F32, tag="ln_sq")
    nc.scalar.activation(sq, mv2[:, :, 1], AF.Sqrt, bias=nc._eps_t[:, :],
                         scale=1.0)
    rs = pool.tile([P, n], F32, tag="ln_rs")
    nc.vector.reciprocal(rs, sq)
    if gate_cols is not None:
        nc.vector.tensor_tensor(out=rs, in0=rs, in1=gate_cols, op=OP.mult)
    return rs


def _rsqrt_newton(nc, pool, v_col, n=1):
    """1/sqrt(v+eps) on DVE only (no ACT tables): seed y0=1/v, then 3
    Newton steps y <- y*(1.5 - 0.5*v*y^2).  Post-residual LN variance
    sits in ~[0.8, 1.6], where this converges to ~1e-4."""
    v = pool.tile([P, n], F32, tag="nw_v")
    nc.vector.tensor_scalar_add(v, v_col, nc._eps_t[:, 0:1])
    y = pool.tile([P, n], F32, tag="nw_y")
    nc.vector.reciprocal(y, v)
    hv = pool.tile([P, n], F32, tag="nw_hv")
    nc.vector.tensor_scalar_mul(hv, v, -0.5)
    for _ in range(3):
        y2 = pool.tile([P, n], F32, tag="nw_y2", bufs=2)
        nc.vector.tensor_tensor(out=y2, in0=y, in1=y, op=OP.mult)
        t = pool.tile([P, n], F32, tag="nw_t", bufs=2)
        nc.vector.tensor_scalar(out=t, in0=y2, scalar1=hv[:, 0:1],
                                scalar2=1.5, op0=OP.mult, op1=OP.add)
        yn = pool.tile([P, n], F32, tag="nw_yn", bufs=2)
        nc.vector.tensor_tensor(out=yn, in0=y, in1=t, op=OP.mult)
        y = yn
    return y


def _build_nc():
    nc = bacc.Bacc("TRN2", target_bir_lowering=False, debug=False,
                   num_devices=N_CORES)

    def din(name, shape, dt=F32):
        return nc.dram_tensor(name, shape, dt, kind="ExternalInput")

    q_d = din("q", [K, D], BF16)
    kv_d = din("kv", [S, D], BF16)
    et_d = din("et", [S, K], BF16)
    gate_d = din("gate", [K, 1])
    qres_d = din("q_res", [KQ, D])
    wq_d = din("wq", [D, DH], BF16)
    wk_d = din("wk", [D, DH], BF16)
    wv_d = din("wv", [D, DH], BF16)
    bq_d = din("bq", [1, DH], BF16)
    bk_d = din("bk", [DH, 1])
    bv_d = din("bv", [DH, 1])
    grow_d = din("growb", [1, K], BF16)
    wo_d = din("wo", [DH, D], BF16)
    w1_d = din("w1", [D, D2], BF16)
    b1_d = din("b1", [1, D2], BF16)
    w2_d = din("w2", [D2, D], BF16)
    b2_d = din("b2", [1, D], BF16)
    out_d = nc.dram_tensor("xq", [KQ, D], F32, kind="ExternalOutput")

    rs_out = [nc.dram_tensor(f"rs_out{i}", [P, D], FP8) for i in range(2)]
    groups = [[0, 1, 2, 3], [4, 5, 6, 7]]

    with tile.TileContext(nc) as tc:
        with (
            tc.tile_pool(name="const", bufs=1) as cpool,
            tc.tile_pool(name="persist", bufs=1) as pp,
            tc.tile_pool(name="dram", bufs=1, space="DRAM") as dpool,
        ):
            # ---- constants ----
            eps_t = cpool.tile([P, 1], F32)
            nc.vector.memset(eps_t, EPS)
            nc._eps_t = eps_t
            ident = cpool.tile([P, P], BF16)
            make_identity(nc, ident)
            ones_row = cpool.tile([1, 512], BF16)
            nc.vector.memset(ones_row, 1.0)
            ones64 = cpool.tile([P, 64], BF16)
            nc.vector.memset(ones64, 1.0)
            gsb = cpool.tile([P, DC], F32)
            nc.sync.dma_start(out=gsb, in_=gate_d.ap().rearrange(
                "(t p) o -> p (t o)", p=P))
            grow_bf = cpool.tile([1, K], BF16)
            nc.sync.dma_start(out=grow_bf, in_=grow_d[:, :])
            bq_bf = cpool.tile([1, DH], BF16)
            nc.sync.dma_start(out=bq_bf, in_=bq_d[:, :])
            bk_col = cpool.tile([P, 2], F32)
            nc.sync.dma_start(out=bk_col, in_=bk_d.ap().rearrange(
                "(c p) o -> p (c o)", p=P))
            bv_col = cpool.tile([P, 2], F32)
            nc.sync.dma_start(out=bv_col, in_=bv_d.ap().rearrange(
                "(c p) o -> p (c o)", p=P))
            b1_bf = cpool.tile([1, D2], BF16)
            nc.sync.dma_start(out=b1_bf, in_=b1_d[:, :])
            b2_bf = cpool.tile([1, D], BF16)
            nc.sync.dma_start(out=b2_bf, in_=b2_d[:, :])

            # ---- persistent activation tensors ----
            qpT = pp.tile([P, 2, K], BF16)       # [2 heads x 64, hp, k]
            kpT = pp.tile([P, 2, S], BF16)
            vp = pp.tile([P, 32, HC * 65], BF16)  # [s%128, s//128, h*65+(hd|one)]
            ctxT = pp.tile([P, 2, K], BF16)

            # ============ q: LN+gate -> PE transpose -> projection ========
            with (
                tc.tile_pool(name="projw", bufs=1) as wpool,
                tc.tile_pool(name="psA", bufs=4, space="PSUM") as psA,
                tc.tile_pool(name="psT", bufs=3, space="PSUM") as psT,
            ):
                wq_bf = wpool.tile([P, DC, DH], BF16)
                nc.gpsimd.dma_start(out=wq_bf, in_=wq_d.ap().rearrange(
                    "(c p) n -> p c n", p=P))
                wk_bf = wpool.tile([P, DC, DH], BF16)
                nc.gpsimd.dma_start(out=wk_bf, in_=wk_d.ap().rearrange(
                    "(c p) n -> p c n", p=P))
                wv_bf = wpool.tile([P, DC, DH], BF16)
                nc.gpsimd.dma_start(out=wv_bf, in_=wv_d.ap().rearrange(
                    "(c p) n -> p c n", p=P))

                def ln_pair(lpool, xts, gate_cols=None):
                    """LN a pair of loaded [128, D] tiles -> two bf16 tiles."""
                    mv2 = lpool.tile([P, 2, 2], F32, tag="ln_mv")
                    for i, xt in enumerate(xts):
                        _ln_stats(nc, lpool, xt, mv2[:, i, :])
                    rs2 = _inv_std(nc, lpool, mv2, 2, gate_cols)
                    return [
                        _ln_norm(nc, lpool, xt, mv2[:, i, 0:1], rs2[:, i:i + 1])
                        for i, xt in enumerate(xts)]

                def transp_tile(xn, dst, copy_eng):
                    tp = psT.tile([P, DC, P], BF16, tag="tp")
                    for dc in range(DC):
                        nc.tensor.transpose(
                            tp[:, dc, :], xn[:, dc * P:(dc + 1) * P], ident)
                    if copy_eng == "s":
                        nc.scalar.copy(dst, tp)
                    else:
                        nc.vector.tensor_copy(dst, tp)

                def q_proj_unit(hp, tb, qT):
                    # q projection chain: psum[2hd, 512 tok] (+rank-1 gate
                    # bias term)
                    ps = psA.tile([P, 512], F32, tag="mm")
                    for dc in range(DC):
                        nc.tensor.matmul(
                            ps[:, :],
                            wq_bf[:, dc, hp * P:(hp + 1) * P],
                            qT[:, dc, tb * 512:(tb + 1) * 512],
                            start=(dc == 0), stop=False)
                    nc.tensor.matmul(
                        ps[:, :], bq_bf[0:1, hp * P:(hp + 1) * P],
                        grow_bf[0:1, tb * 512:(tb + 1) * 512],
                        start=False, stop=True)
                    nc.scalar.activation(
                        qpT[:, hp, tb * 512:(tb + 1) * 512], ps[:, :],
                        AF.Identity)

                def kv_proj_unit(sg, hp, sb_, which, kvT, vpT):
                    # one k- or v-projection chain (bias folded into drain)
                    ssl = slice(sb_ * 512, (sb_ + 1) * 512)
                    w_bf = wk_bf if which == "k" else wv_bf
                    ps = psA.tile([P, 512], F32, tag="mm")
                    for dc in range(DC):
                        nc.tensor.matmul(
                            ps[:, :],
                            w_bf[:, dc, hp * P:(hp + 1) * P],
                            kvT[:, dc, ssl],
                            start=(dc == 0), stop=(dc == DC - 1))
                    if which == "k":
                        osl = slice(sg * 1024 + sb_ * 512,
                                    sg * 1024 + (sb_ + 1) * 512)
                        nc.scalar.activation(
                            kpT[:, hp, osl], ps[:, :], AF.Identity,
                            bias=bk_col[:, hp:hp + 1], scale=1.0)
                    else:
                        nc.scalar.activation(
                            vpT[:, hp, ssl], ps[:, :], AF.Identity,
                            bias=bv_col[:, hp:hp + 1], scale=1.0)

                def vp_unit(sg, hp, vpT):
                    # vp natural layout [s, hd] per head (+ ones column):
                    # PE-transpose vpT chunks, then strided inserts.
                    tp = psT.tile([P, DC, P], BF16, tag="tp")
                    for dc in range(DC):
                        nc.tensor.transpose(
                            tp[:, dc, :],
                            vpT[:, hp, dc * P:(dc + 1) * P], ident)
                    for half in range(2):
                        h = hp * 2 + half
                        if half == 0:
                            nc.vector.tensor_copy(
                                vp[:, sg * 8:(sg + 1) * 8,
                                   h * 65:h * 65 + 64],
                                tp[:, :, half * 64:half * 64 + 64])
                        else:
                            nc.scalar.copy(
                                vp[:, sg * 8:(sg + 1) * 8,
                                   h * 65:h * 65 + 64],
                                tp[:, :, half * 64:half * 64 + 64])

                # q LN/transposes, then kv by s-group.  The previous
                # group's projection chains are emitted BETWEEN the next
                # group's LN/transpose pairs so the PE never starves (and
                # stays at full p-state) while the DVE runs LN stats.
                qT = wpool.tile([P, DC, K], BF16, tag="qT")
                with (
                    tc.tile_pool(name="lnq", bufs=4) as lpool,
                    tc.tile_pool(name="lnkv_big", bufs=2) as kbig,
                ):
                    for tp_ in range(4):
                        xts = []
                        for i in range(2):
                            t = tp_ * 2 + i
                            qt = lpool.tile([P, D], BF16, tag=f"ln_in{i}")
                            nc.gpsimd.dma_start(
                                out=qt, in_=q_d[t * P:(t + 1) * P, :])
                            xts.append(qt)
                        qns = ln_pair(lpool, xts,
                                      gate_cols=gsb[:, tp_ * 2:tp_ * 2 + 2])
                        for i in range(2):
                            t = tp_ * 2 + i
                            transp_tile(qns[i], qT[:, :, t * P:(t + 1) * P],
                                        "s")
                    for h in range(HC):
                        nc.vector.memset(vp[:, :, h * 65 + 64:h * 65 + 65], 1.0)
                    pending = [lambda hp=hp, tb=tb: q_proj_unit(hp, tb, qT)
                               for hp in range(2) for tb in range(2)]
                    for sg in range(4):
                        kvT = kbig.tile([P, DC, 1024], BF16, tag="kvT")
                        vpT = kbig.tile([P, 2, 1024], BF16, tag="vpT")
                        for tp_ in range(4):
                            xts = []
                            for i in range(2):
                                t = tp_ * 2 + i
                                st_ = sg * 1024 + t * P
                                xt = lpool.tile([P, D], BF16, tag=f"ln_in{i}")
                                nc.gpsimd.dma_start(
                                    out=xt, in_=kv_d[st_:st_ + P, :])
                                xts.append(xt)
                            xns = ln_pair(lpool, xts)
                            for i in range(2):
                                t = tp_ * 2 + i
                                transp_tile(
                                    xns[i], kvT[:, :, t * P:(t + 1) * P],
                                    "s")
                            n_emit = (len(pending) + 3 - tp_) // (4 - tp_)
                            for u in pending[:n_emit]:
                                u()
                            pending = pending[n_emit:]
                        assert not pending
                        pending = []
                        for hp in range(2):
                            for sb_ in range(2):
                                for which in ("k", "v"):
                                    pending.append(
                                        lambda sg=sg, hp=hp, sb_=sb_,
                                        which=which, kvT=kvT, vpT=vpT:
                                        kv_proj_unit(sg, hp, sb_, which,
                                                     kvT, vpT))
                            pending.append(
                                lambda sg=sg, hp=hp, vpT=vpT:
                                vp_unit(sg, hp, vpT))
                        # order: k/v chains for hp then its vp transpose
                    for u in pending:
                        u()

            # tail weights: pool ring is idle once q/kv loads are done;
            # these loads overlap the start of attention.  The wt pool is
            # entered only now so its 76KB/partition comes from the freed
            # prologue zones instead of shrinking them.
            wt_cm = tc.tile_pool(name="wt", bufs=1)
            wt = wt_cm.__enter__()
            wo_bf = wt.tile([P, 2, D], BF16)
            nc.gpsimd.dma_start(out=wo_bf, in_=wo_d.ap().rearrange(
                "(c p) n -> p c n", p=P))
            w1_bf = wt.tile([P, DC, D2], BF16)
            nc.gpsimd.dma_start(out=w1_bf, in_=w1_d.ap().rearrange(
                "(c p) n -> p c n", p=P))
            w2_bf = wt.tile([P, D2 // P, D], BF16)
            nc.gpsimd.dma_start(out=w2_bf, in_=w2_d.ap().rearrange(
                "(c p) n -> p c n", p=P))
            qres_sb = wt.tile([P, 2, D], F32)
            nc.gpsimd.dma_start(out=qres_sb, in_=qres_d.ap().rearrange(
                "(t p) d -> p t d", p=P))
            x_sb = wt.tile([P, 2, D], F32)
            xfT = wt.tile([P, DC, KQ], BF16)

            # ======================= attention ==========================
            # loop kb (k halves) -> sc (s tiles) -> hp (head pairs);
            # the bias-exp block streams from HBM per (kb, sc).  After each
            # kb, the out-proj partial for that k-half is computed and its
            # ReduceScatter launched (overlapping the next kb / the FFN).
            with (
                tc.tile_pool(name="att", bufs=8) as apool,
                tc.tile_pool(name="ets", bufs=6) as espool,
                tc.tile_pool(name="attr", bufs=2) as rpool,
                tc.tile_pool(name="ysb", bufs=1) as ypool,
                tc.tile_pool(name="psS", bufs=2, space="PSUM") as psS,
                tc.tile_pool(name="psPV", bufs=1, space="PSUM") as psPV,
            ):
                def qk_exp_mult(kb, sc):
                    # scores -> exp -> bias-multiply for one s-tile; the
                    # returned `at` tiles live in the 6-deep ring until
                    # their PV matmuls consume them.
                    ksl = slice(kb * 512, (kb + 1) * 512)
                    et_blk = espool.tile([P, 512], BF16, tag="et")
                    nc.sync.dma_start(
                        out=et_blk,
                        in_=et_d.ap()[sc * P:(sc + 1) * P, ksl])
                    ats = []
                    for hp in range(2):
                        sps = psS.tile([P, 1024], F32, tag="sps")
                        nc.tensor.matmul(
                            sps[:, 0:512],
                            kpT[0:64, hp, sc * P:(sc + 1) * P],
                            qpT[0:64, hp, ksl],
                            start=True, stop=True, tile_position=(0, 0))
                        nc.tensor.matmul(
                            sps[:, 512:1024],
                            kpT[64:128, hp, sc * P:(sc + 1) * P],
                            qpT[64:128, hp, ksl],
                            start=True, stop=True, tile_position=(64, 0))
                        eq = apool.tile([P, 1024], BF16, tag="eq")
                        nc.scalar.activation(eq, sps[:, :], AF.Exp)
                        at = apool.tile([P, 1024], BF16, tag="at")
                        et_v = et_blk[:, :].rearrange(
                            "p (o f) -> p o f", o=1).broadcast_to(
                            [P, 2, 512])
                        nc.vector.tensor_tensor(
                            out=at[:, :].rearrange("p (o f) -> p o f", o=2),
                            in0=eq[:, :].rearrange("p (o f) -> p o f", o=2),
                            in1=et_v, op=OP.mult)
                        ats.append(at)
                    return ats

                pre_ats = None
                for kb in range(K // 512):
                    ksl = slice(kb * 512, (kb + 1) * 512)
                    pvs = [psPV.tile([65, 512], F32, tag=f"pv{h}",
                                     name=f"pv_{kb}_{h}")
                           for h in range(HC)]
                    for sc in range(S // P):
                        if pre_ats is not None and sc < len(pre_ats):
                            ats = pre_ats[sc]
                        else:
                            ats = qk_exp_mult(kb, sc)
                        for hp in range(2):
                            at = ats[hp]
                            he = hp * 2
                            ho = hp * 2 + 1
                            nc.tensor.matmul(
                                pvs[he][:, :],
                                vp[:, sc, he * 65:(he + 1) * 65],
                                at[:, 0:512],
                                start=(sc == 0), stop=(sc == S // P - 1))
                            nc.tensor.matmul(
                                pvs[ho][:, :],
                                vp[:, sc, ho * 65:(ho + 1) * 65],
                                at[:, 512:1024],
                                start=(sc == 0), stop=(sc == S // P - 1))
                    # prefetch the NEXT K-half's first scores/exp/multiply
                    # so the scalar engine keeps running through the
                    # epilogue below (its PE work sits ahead of the
                    # epilogue matmuls in the queue).
                    if kb == 0:
                        pre_ats = [qk_exp_mult(1, s) for s in range(2)]
                    # denominators: spread the 4 heads onto partitions
                    # {0,32,64,96} so one reciprocal covers all of them;
                    # broadcast 1/den across 64 partitions with a 1-row PE
                    # matmul whose psum borrows a freed score-ring slot.
                    last = kb == K // 512 - 1
                    dall = rpool.tile([97, 512], F32, tag="dall")
                    for h in range(HC):
                        if h % 2 == 1:
                            nc.scalar.copy(dall[32 * h:32 * h + 1, :],
                                           pvs[h][64:65, :])
                        else:
                            nc.vector.tensor_copy(dall[32 * h:32 * h + 1, :],
                                                  pvs[h][64:65, :])
                    rden = rpool.tile([97, 512], BF16, tag="rden")
                    with nc.allow_low_precision(
                            reason="softmax denom reciprocal in bf16; "
                                   "ctx is bf16 anyway"):
                        nc.vector.reciprocal(rden, dall[:, :])
                    for h in range(HC):
                        pv = pvs[h]
                        pb = (h % 2) * 64
                        hp = h // 2
                        rps = psS.tile([64, 512], F32, tag="sps",
                                       name=f"rps_{kb}_{h}")
                        nc.tensor.matmul(rps[:, :],
                                         ones64[32 * h:32 * h + 1, :],
                                         rden[32 * h:32 * h + 1, :],
                                         start=True, stop=True,
                                         tile_position=(32 * h, 0))
                        rrs = rpool.tile([64, 512], BF16, tag="rrs")
                        if last and h % 2 == 1:
                            nc.scalar.copy(rrs, rps[:, :])
                        else:
                            nc.vector.tensor_copy(rrs, rps[:, :])
                        nc.vector.tensor_tensor(
                            out=ctxT[pb:pb + 64, hp, ksl],
                            in0=pv[0:64, :], in1=rrs, op=OP.mult)
                    # ---- out-proj partial for this k-half + ReduceScatter;
                    # psums borrow the freed pv banks (same tag rings).
                    y_sb = ypool.tile([P, 4, D], FP8, tag="y")
                    rs_in = dpool.tile([512, D], FP8, tag=f"rsin{kb}")
                    for tb in range(4):
                        tsl = slice(kb * 512 + tb * P, kb * 512 + (tb + 1) * P)
                        for db in range(D // 512):
                            dsl = slice(db * 512, (db + 1) * 512)
                            ps = psPV.tile([P, 512], F32,
                                           tag=f"pv{(tb * 2 + db) % 4}",
                                           name=f"op_{kb}_{tb}_{db}")
                            for cc in range(2):
                                nc.tensor.matmul(
                                    ps[:, :],
                                    ctxT[:, cc, tsl],
                                    wo_bf[:, cc, dsl],
                                    start=(cc == 0), stop=(cc == 1))
                            if last and db == 1:
                                nc.scalar.copy(y_sb[:, tb, dsl], ps[:, :])
                            else:
                                nc.vector.tensor_copy(y_sb[:, tb, dsl],
                                                      ps[:, :])
                        nc.gpsimd.dma_start(
                            out=rs_in[tb * P:(tb + 1) * P, :],
                            in_=y_sb[:, tb, :])
                    nc.gpsimd.collective_compute(
                        "ReduceScatter", OP.add, replica_groups=groups,
                        ins=[rs_in.opt()], outs=[rs_out[kb].ap().opt()])

            # ====== residual + LN_f + FFN per k-half (kt0 under RS1) ====
            # psFX (4 banks) lands in the freed score-ring zone so FFN1 can
            # start during the kb=1 epilogue; psH2 reuses the pv zone.
            with (
                tc.tile_pool(name="ffn", bufs=1) as fp,
                tc.tile_pool(name="fstream", bufs=2) as fs,
                tc.tile_pool(name="psFX", bufs=1, space="PSUM") as psFX,
                tc.tile_pool(name="psH2", bufs=1, space="PSUM") as psH2,
            ):
                h1T = fp.tile([P, D2 // P, KQ], BF16)
                o_sb = fp.tile([P, 2, D], F32)
                for kt in range(2):
                    rs_sb = fs.tile([P, D], FP8, tag="rs")
                    nc.sync.dma_start(out=rs_sb, in_=rs_out[kt].ap())
                    nc.vector.tensor_tensor(out=x_sb[:, kt, :], in0=rs_sb,
                                            in1=qres_sb[:, kt, :], op=OP.add)
                    mv2 = fs.tile([P, 1, 2], F32, tag="ln_mv")
                    _ln_stats(nc, fs, x_sb[:, kt, :], mv2[:, 0, :])
                    rs1c = _rsqrt_newton(nc, fs, mv2[:, 0, 1:2])
                    xn = _ln_norm(nc, fs, x_sb[:, kt, :], mv2[:, 0, 0:1],
                                  rs1c[:, 0:1])
                    tpx = psFX.tile([P, DC, P], BF16, tag="tpx")
                    for dc in range(DC):
                        nc.tensor.transpose(
                            tpx[:, dc, :], xn[:, dc * P:(dc + 1) * P], ident)
                    nc.scalar.copy(xfT[:, :, kt * P:(kt + 1) * P], tpx)
                    # FFN1 flipped: xfT chunks stationary, w1 streams
                    h1 = fs.tile([P, 4, 512], BF16, tag="h1")
                    for hb in range(4):
                        ps = psFX.tile([P, 512], F32, tag="f", bufs=3)
                        hsl = slice(hb * 512, (hb + 1) * 512)
                        for dc in range(DC):
                            nc.tensor.matmul(
                                ps[:, :], xfT[:, dc, kt * P:(kt + 1) * P],
                                w1_bf[:, dc, hsl],
                                start=(dc == 0), stop=False)
                        nc.tensor.matmul(
                            ps[:, :], ones_row[0:1, 0:P],
                            b1_bf[0:1, hsl], start=False, stop=True)
                        nc.scalar.activation(h1[:, hb, :], ps[:, :], AF.Gelu)
                    tph = psH2.tile([P, D2 // P, P], BF16, tag="tph")
                    h1f = h1[:, :, :].rearrange("p a b -> p (a b)")
                    for hc in range(D2 // P):
                        nc.tensor.transpose(
                            tph[:, hc, :], h1f[:, hc * P:(hc + 1) * P], ident)
                    nc.scalar.copy(h1T[:, 0:8, kt * P:(kt + 1) * P],
                                   tph[:, 0:8, :])
                    nc.vector.tensor_copy(h1T[:, 8:16, kt * P:(kt + 1) * P],
                                          tph[:, 8:16, :])
                    # FFN2: accumulate over hc chunks, both D halves live
                    ps0 = psH2.tile([P, 512], F32, tag="o0")
                    ps1 = psH2.tile([P, 512], F32, tag="o1")
                    for hc in range(D2 // P):
                        for db, ps in ((0, ps0), (1, ps1)):
                            nc.tensor.matmul(
                                ps[:, :], h1T[:, hc, kt * P:(kt + 1) * P],
                                w2_bf[:, hc, db * 512:(db + 1) * 512],
                                start=(hc == 0), stop=False)
                    for db, ps in ((0, ps0), (1, ps1)):
                        dsl = slice(db * 512, (db + 1) * 512)
                        nc.tensor.matmul(
                            ps[:, :], ones_row[0:1, 0:P],
                            b2_bf[0:1, dsl], start=False, stop=True)
                        nc.vector.tensor_tensor(out=o_sb[:, kt, dsl],
                                                in0=ps[:, :],
                                                in1=x_sb[:, kt, dsl],
                                                op=OP.add)
                        nc.sync.dma_start(
                            out=out_d.ap()[kt * P:(kt + 1) * P, dsl],
                            in_=o_sb[:, kt, dsl])
            wt_cm.__exit__(None, None, None)

    nc.compile()
    return nc


def _get_nc():
    if "nc" not in _CACHE:
        _CACHE["nc"] = _build_nc()
    return _CACHE["nc"]


def _softplus(x):
    return float(np.log1p(np.exp(np.float64(x))))


def kernel(**inputs):
    f = lambda name: np.ascontiguousarray(np.asarray(inputs[name], np.float32))
    q = f("q"); kv = f("kv"); ab = f("attn_bias"); ob = f("obs_bias")
    density = f("density")
    c1 = _softplus(inputs["dist_raw"])
    c2 = _softplus(inputs["obs_raw"])
    tg = float(np.tanh(np.float64(np.asarray(inputs["dens_raw"], np.float64))))
    gate = (1.0 + tg * density).astype(np.float32)       # [B, K]

    ln_q_w = f("ln_q_w"); ln_q_b = f("ln_q_b")
    ln_kv_w = f("ln_kv_w"); ln_kv_b = f("ln_kv_b")
    ln_f_w = f("ln_f_w"); ln_f_b = f("ln_f_b")
    scale = np.float32(HD ** -0.5)
    wq = scale * ln_q_w[:, None] * f("wq")
    bq = scale * (ln_q_b @ f("wq") + f("bq"))
    wk = ln_kv_w[:, None] * f("wk"); bk = ln_kv_b @ f("wk") + f("bk")
    wv = ln_kv_w[:, None] * f("wv"); bv = ln_kv_b @ f("wv") + f("bv")
    w1 = ln_f_w[:, None] * f("w1"); b1 = ln_f_b @ f("w1") + f("b1")
    wo = f("wo"); bo = f("bo"); w2 = f("w2"); b2 = f("b2")

    # host-side: exp of the gated bias sum, transposed to [S, K] bf16
    et_host = []
    for b in range(B):
        cb = (c1 * ab[b] + c2 * ob[b]) * gate[b][:, None]   # [K, S]
        et_host.append(np.ascontiguousarray(
            np.exp(cb.T).astype(ml_dtypes.bfloat16)))        # [S, K]

    cont = np.ascontiguousarray
    bf = lambda a: np.ascontiguousarray(np.asarray(a, dtype=ml_dtypes.bfloat16))
    in_maps = []
    row_maps = []
    for c in range(N_CORES):
        b, r = divmod(c, 4)
        hs = slice(r * DH, (r + 1) * DH)
        rows = np.r_[r * P:(r + 1) * P, 512 + r * P:512 + (r + 1) * P]
        row_maps.append((b, rows))
        in_maps.append({
            "q": bf(q[b]), "kv": bf(kv[b]),
            "et": et_host[b],
            "gate": cont(gate[b][:, None]),
            "growb": bf(gate[b][None, :]),
            "q_res": cont(q[b][rows] + bo[None, :]),
            "wq": bf(wq[:, hs]), "wk": bf(wk[:, hs]), "wv": bf(wv[:, hs]),
            "bq": bf(bq[None, hs]), "bk": cont(bk[hs, None]),
            "bv": cont(bv[hs, None]),
            "wo": bf(wo[hs, :]), "w1": bf(w1), "b1": bf(b1[None, :]),
            "w2": bf(w2), "b2": bf(b2[None, :]),
        })

    global _last_in_maps
    _last_in_maps = in_maps
    nc = _get_nc()
    res = bass_utils.run_bass_kernel_spmd(
        nc, in_maps, core_ids=list(range(N_CORES)))
    out = np.empty((B, K, D), np.float32)
    for c in range(N_CORES):
        b, rows = row_maps[c]
        out[b][rows] = res.results[c]["xq"]
    return out
